# revision 21
# baseline (speedup 1.0000x reference)
"""Trainium2 Bass kernel for nn_Encoder (GRU + input attention).

Shapes (hardcoded): B=32, T=128, N=256, H=512; 8 NeuronCores, batch
sharded 4 examples/core.

Math (matching the reference):
  hs = GRU scan over T steps (Keras GRUCell, reset_after=True, gates z,r,h)
  score_x[b,n,u] = sum_t data[b,t,n] w1_w[t,u] + w1_b[u]
  hp[t,b,u]     = hs[t,b,:] @ w2_w + w2_b[u]
  score[t,b,n]  = sum_u v[u] tanh(score_x[b,n,u] + hp[t,b,u])   (+v_b: softmax-invariant)
  alpha = softmax_n(score);  out[b,t,:] = data[b,t,:] * alpha[(b*T+t)//B, (b*T+t)%B, :]

Per-core layout: u (or H-chunks) on partitions. The recurrent matmul keeps
R chunks stationary (fp16, FWL) and streams h^T (128,4) slices; gate adds for
z/r are folded into the PSUM accumulation via an identity-matmul; sigmoid is
computed as (1+tanh(x/2))/2 so tanh+exp live in one activation table set.
"""

import numpy as np

B, T, N, H = 32, 128, 256, 512
NC = 8           # cores
BL = B // NC     # batch per core (4)
H3 = 3 * H

_CACHE = {}
DEBUG = False


def _build():
    import concourse.bass as bass
    import concourse.bacc as bacc
    import concourse.tile as tile
    import concourse.mybir as mybir

    f16 = mybir.dt.float16
    f32 = mybir.dt.float32
    Alu = mybir.AluOpType
    Act = mybir.ActivationFunctionType

    nc = bacc.Bacc("TRN2", target_bir_lowering=False, debug=False)

    # ---- dram I/O ----
    d_data16 = nc.dram_tensor("data16", [BL, T, N], f16, kind="ExternalInput")
    d_dataout = nc.dram_tensor("dataout", [4, 128, N], f32, kind="ExternalInput")
    d_h016 = nc.dram_tensor("h016", [BL, H], f16, kind="ExternalInput")
    d_R = nc.dram_tensor("R_l", [128, 48, 128], f16, kind="ExternalInput")
    d_K = nc.dram_tensor("K_l", [128, 2, 12, 128], f16, kind="ExternalInput")
    d_w1 = nc.dram_tensor("w1_l", [128, 128], f16, kind="ExternalInput")
    d_w2 = nc.dram_tensor("w2_l", [128, 4, 128], f16, kind="ExternalInput")
    d_vbuf = nc.dram_tensor("vbuf", [128, 257], f16, kind="ExternalInput")
    d_ident = nc.dram_tensor("ident", [128, 128], f16, kind="ExternalInput")
    d_bzr = nc.dram_tensor("bias_zr", [128, 8], f32, kind="ExternalInput")
    d_bh = nc.dram_tensor("bias_h", [128, 4], f32, kind="ExternalInput")
    d_brech = nc.dram_tensor("brech_rep", [128, 16, T], f16, kind="ExternalInput")
    d_bu = nc.dram_tensor("bias_u", [128, 1], f32, kind="ExternalInput")
    d_out = nc.dram_tensor("out", [4, 128, N], f32, kind="ExternalOutput")
    if DEBUG:
        d_hs = nc.dram_tensor("hs_dump", [128, T + 1, 16], f16,
                              kind="ExternalOutput")
        d_sxd = nc.dram_tensor("sx_dump", [128, BL, N], f16,
                               kind="ExternalOutput")
        d_alp = nc.dram_tensor("alpha_dump", [4, 128, N], f16,
                               kind="ExternalOutput")
        d_amx = nc.dram_tensor("addmx_dump", [128, 48, T], f16,
                               kind="ExternalOutput")
        d_xhd = nc.dram_tensor("xh_dump", [128, 16, T], f16,
                               kind="ExternalOutput")

    LAG_E = 8    # e-tile tanh lag behind the scan
    LAG_S = 9    # score/softmax lag

    with tile.TileContext(nc) as tc:
        with (
            tc.tile_pool(name="const", bufs=1) as cpool,
            tc.tile_pool(name="work", bufs=3) as wpool,
            tc.tile_pool(name="ebuf", bufs=3) as epool,
            tc.tile_pool(name="hpbuf", bufs=2) as hppool,
            tc.tile_pool(name="mh", bufs=2, space="PSUM") as mhpool,
            tc.tile_pool(name="bigps", bufs=2, space="PSUM") as bpool,
            tc.tile_pool(name="hpps", bufs=2, space="PSUM") as hpspool,
        ):
            # ---- persistent tiles ----
            t_R = cpool.tile([128, 48, 128], f16)
            t_K = cpool.tile([128, 2, 12, 128], f16)
            t_w1 = cpool.tile([128, 128], f16)
            t_w2 = cpool.tile([128, 4, 128], f16)
            t_vbuf = cpool.tile([128, 257], f16)
            t_ident = cpool.tile([128, 128], f16)
            t_bzr = cpool.tile([128, 8], f32)
            t_bh = cpool.tile([128, 4], f32)
            t_bu = cpool.tile([128, 1], f32)
            t_d16 = [cpool.tile([128, N], f16, tag=f"d16_{b}", name=f"d16_{b}")
                     for b in range(BL)]
            t_dT = cpool.tile([128, 2, BL, 128], f16)      # dataT [p, nc, b, t]
            t_h0 = cpool.tile([BL, H], f16)
            t_addmx = cpool.tile([128, 48, T], f16)        # [mx_zr' | b_rec_h] per t
            t_xh = cpool.tile([128, 16, T], f16)           # xh' per t
            t_sx = cpool.tile([128, BL, N], f16)           # score_x' per b
            t_hs = cpool.tile([128, T + 1, 16], f16)       # h^T packed, slot t+1 = hs[t]
            t_alpha = [cpool.tile([128, N], f16, tag=f"alpha_{k}", name=f"alpha_{k}")
                       for k in range(4)]
            t_ssum = cpool.tile([128, 1], f32)
            t_rinv = cpool.tile([128, 1], f32)

            # ---- DMA in ----
            for b in range(BL):
                nc.sync.dma_start(out=t_d16[b][:, :], in_=d_data16.ap()[b, :, :])
            nc.sync.dma_start(out=t_R[:, :, :], in_=d_R.ap()[:, :, :])
            nc.sync.dma_start(out=t_K[:, :, :, :], in_=d_K.ap()[:, :, :, :])
            nc.sync.dma_start(out=t_w1[:, :], in_=d_w1.ap()[:, :])
            nc.sync.dma_start(out=t_w2[:, :, :], in_=d_w2.ap()[:, :, :])
            nc.sync.dma_start(out=t_vbuf[:, :], in_=d_vbuf.ap()[:, :])
            nc.sync.dma_start(out=t_ident[:, :], in_=d_ident.ap()[:, :])
            nc.sync.dma_start(out=t_bzr[:, :], in_=d_bzr.ap()[:, :])
            nc.sync.dma_start(out=t_bh[:, :], in_=d_bh.ap()[:, :])
            nc.sync.dma_start(out=t_addmx[:, 32:48, :], in_=d_brech.ap()[:, :, :])
            nc.sync.dma_start(out=t_bu[:, :], in_=d_bu.ap()[:, :])
            nc.sync.dma_start(out=t_h0[:, :], in_=d_h016.ap()[:, :])

            # ---- prologue: h0^T into hs slot 0 ----
            for j in range(4):
                ps = bpool.tile([128, 128], f16, tag="bigps")
                nc.tensor.transpose(ps[:, 0:BL], t_h0[0:BL, 128 * j:128 * (j + 1)],
                                    t_ident[0:BL, 0:BL])
                nc.vector.tensor_copy(t_hs[:, 0, 4 * j:4 * j + 4], ps[:, 0:BL])

            # ---- prologue: data^T  [p, nc, b, t] ----
            for b in range(BL):
                for n2 in range(2):
                    ps = bpool.tile([128, 128], f16, tag="bigps")
                    nc.tensor.transpose(ps[:, :], t_d16[b][:, 128 * n2:128 * (n2 + 1)],
                                        t_ident[:, :])
                    nc.vector.tensor_copy(t_dT[:, n2, b, :], ps[:, :])

            # ---- prologue: mx = data @ K (+biases), scattered per t ----
            for uc in range(12):
                ps = bpool.tile([128, BL, 128], f32, tag="bigps")
                for n2 in range(2):
                    nc.tensor.matmul(ps[:, :, :], t_K[:, n2, uc, :],
                                     t_dT[:, n2, :, :],
                                     start=(n2 == 0), stop=(n2 == 1))
                g, j = divmod(uc, 4)
                # psum free order is (b, t); dest free dims (b-within-col, t)
                if g < 2:
                    nc.scalar.activation(
                        t_addmx[:, 4 * uc:4 * uc + 4, :], ps[:, :, :],
                        Act.Identity, bias=t_bzr[:, uc:uc + 1])
                else:
                    nc.scalar.activation(
                        t_xh[:, 4 * j:4 * j + 4, :], ps[:, :, :],
                        Act.Identity, bias=t_bh[:, j:j + 1])

            # ---- prologue: score_x' ----
            for b in range(BL):
                ps = bpool.tile([128, N], f32, tag="bigps")
                nc.tensor.matmul(ps[:, :], t_w1[:, :], t_d16[b][:, :],
                                 start=True, stop=True)
                nc.scalar.activation(t_sx[:, b, :], ps[:, :],
                                     Act.Identity, bias=t_bu[:, :])

            # ---- helpers for the lagged attention pipeline ----
            hp_tiles = {}  # block index -> sbuf tile [128, 32] f32

            def emit_hp_block(blk):
                t0 = 8 * blk
                ps = hpspool.tile([128, 32], f32)
                for hc in range(4):
                    nc.tensor.matmul(ps[:, :], t_w2[:, hc, :],
                                     t_hs[:, t0 + 1:t0 + 9, 4 * hc:4 * hc + 4],
                                     start=(hc == 0), stop=(hc == 3))
                hp = hppool.tile([128, 32], f32)
                nc.scalar.activation(hp[:, :], ps[:, :], Act.Identity)
                hp_tiles[blk] = hp

            e_tiles = {}  # ta -> e tile [128, BL, N] f16

            def emit_e(ta):
                hp = hp_tiles[ta // 8]
                tl = ta % 8
                et = epool.tile([128, BL, N], f16, tag="etile")
                for b in range(BL):
                    nc.scalar.activation(et[:, b, :], t_sx[:, b, :], Act.Tanh,
                                         bias=hp[:, 4 * tl + b:4 * tl + b + 1])
                e_tiles[ta] = et

            score_ps = {}  # group (8 steps) -> psum tile [128, N] f32

            def emit_score_mm(ta):
                et = e_tiles.pop(ta)
                g, tl = divmod(ta, 8)
                if tl == 0:
                    score_ps[g] = bpool.tile([128, N], f32, tag="bigps",
                                             name=f"scps_{g}")
                ps = score_ps[g]
                c32 = ta % 32
                for l in range(BL):
                    co = 128 - (4 * c32 + l)
                    nc.tensor.matmul(ps[:, :], t_vbuf[:, co:co + 128], et[:, l, :],
                                     start=(tl == 0 and l == 0),
                                     stop=(tl == 7 and l == BL - 1),
                                     skip_group_check=True)

            def emit_softmax_group(g):
                ps = score_ps.pop(g)
                p0 = 32 * (g % 4)
                ex = wpool.tile([128, N], f16, tag="expv")
                nc.scalar.activation(ex[p0:p0 + 32, :], ps[p0:p0 + 32, :], Act.Exp,
                                     accum_out=t_ssum[p0:p0 + 32, :])
                nc.vector.reciprocal(t_rinv[p0:p0 + 32, :], t_ssum[p0:p0 + 32, :])
                nc.vector.tensor_scalar(
                    t_alpha[g // 4][p0:p0 + 32, :], ex[p0:p0 + 32, :],
                    t_rinv[p0:p0 + 32, :], None, Alu.mult)

            # ---- main scan loop ----
            for t in range(T):
                if t % 8 == 0 and t >= 8:
                    emit_hp_block(t // 8 - 1)

                mh = mhpool.tile([128, 48], f32)
                # bias/mx seed: writes all 48 cols, start=True clears the bank
                nc.tensor.matmul(mh[:, 0:48], t_ident[:, :], t_addmx[:, :, t],
                                 start=True, stop=False, skip_group_check=True)
                # z/r chunks first, then h chunks — all accumulate
                for uc in range(12):
                    for kc in range(4):
                        nc.tensor.matmul(mh[:, 4 * uc:4 * uc + 4],
                                         t_R[:, 12 * kc + uc, :],
                                         t_hs[:, t, 4 * kc:4 * kc + 4],
                                         start=False,
                                         stop=(uc == 11 and kc == 3),
                                         skip_group_check=True)

                # gates
                tz = wpool.tile([128, 16], f16, tag="tz")
                tr = wpool.tile([128, 16], f16, tag="tr")
                zg = wpool.tile([128, 16], f16, tag="zg")
                rg = wpool.tile([128, 16], f16, tag="rg")
                t2 = wpool.tile([128, 16], f16, tag="t2")
                t3 = wpool.tile([128, 16], f16, tag="t3")
                cg = wpool.tile([128, 16], f16, tag="cg")
                dg = wpool.tile([128, 16], f16, tag="dg")
                mg = wpool.tile([128, 16], f16, tag="mg")

                nc.scalar.activation(tz[:, :], mh[:, 0:16], Act.Tanh, scale=0.5)
                nc.scalar.activation(tr[:, :], mh[:, 16:32], Act.Tanh, scale=0.5)
                nc.vector.tensor_scalar(zg[:, :], tz[:, :], 1.0, 0.5,
                                        Alu.add, Alu.mult)
                nc.vector.tensor_scalar(rg[:, :], tr[:, :], 1.0, 0.5,
                                        Alu.add, Alu.mult)
                nc.vector.tensor_tensor(t2[:, :], rg[:, :], mh[:, 32:48], Alu.mult)
                nc.vector.tensor_tensor(t3[:, :], t2[:, :], t_xh[:, :, t], Alu.add)
                nc.scalar.activation(cg[:, :], t3[:, :], Act.Tanh)
                nc.vector.tensor_tensor(dg[:, :], t_hs[:, t, :], cg[:, :],
                                        Alu.subtract)
                nc.vector.tensor_tensor(mg[:, :], zg[:, :], dg[:, :], Alu.mult)
                nc.vector.tensor_tensor(t_hs[:, t + 1, :], cg[:, :], mg[:, :],
                                        Alu.add)

                # lagged attention
                if t >= LAG_E:
                    emit_e(t - LAG_E)
                if t >= LAG_S:
                    emit_score_mm(t - LAG_S)
                    if (t - LAG_S) % 8 == 7:
                        emit_softmax_group((t - LAG_S) // 8)

            # ---- attention epilogue ----
            emit_hp_block(15)
            for ta in range(T - LAG_E, T):
                emit_e(ta)
            for ta in range(T - LAG_S, T):
                emit_score_mm(ta)
                if ta % 8 == 7:
                    emit_softmax_group(ta // 8)

            if DEBUG:
                nc.sync.dma_start(out=d_hs.ap()[:, :, :], in_=t_hs[:, :, :])
                nc.sync.dma_start(out=d_sxd.ap()[:, :, :], in_=t_sx[:, :, :])
                for k in range(4):
                    nc.sync.dma_start(out=d_alp.ap()[k, :, :],
                                      in_=t_alpha[k][:, :])
                nc.sync.dma_start(out=d_amx.ap()[:, :, :], in_=t_addmx[:, :, :])
                nc.sync.dma_start(out=d_xhd.ap()[:, :, :], in_=t_xh[:, :, :])

            # ---- final out = data * alpha ----
            for k in range(4):
                dt_ = wpool.tile([128, N], f32, tag="dmul")
                ot = wpool.tile([128, N], f32, tag="omul")
                nc.sync.dma_start(out=dt_[:, :], in_=d_dataout.ap()[k, :, :])
                nc.vector.tensor_tensor(ot[:, :], dt_[:, :], t_alpha[k][:, :],
                                        Alu.mult)
                nc.sync.dma_start(out=d_out.ap()[k, :, :], in_=ot[:, :])

    nc.compile()
    return nc


def _prep_inputs(data, h0, gru_kernel, gru_rkernel, gru_bias,
                 w1_w, w1_b, w2_w, w2_b, v_w, v_b):
    f16 = np.float16
    f32 = np.float32

    R16 = gru_rkernel.astype(f16)                     # (512, 1536)
    R_l = np.ascontiguousarray(
        R16.reshape(4, 128, 12, 128).transpose(1, 0, 2, 3)
    ).reshape(128, 48, 128)
    K16 = gru_kernel.astype(f16)                      # (256, 1536)
    K_l = np.ascontiguousarray(
        K16.reshape(2, 128, 12, 128).transpose(1, 0, 2, 3))
    w1_l = w1_w.astype(f16)                           # (128, 128)
    w2_l = np.ascontiguousarray(
        w2_w.astype(f16).reshape(4, 128, 128).transpose(1, 0, 2))
    vbuf = np.zeros((128, 257), f16)
    vbuf[:, 128] = v_w[:, 0].astype(f16)
    ident = np.eye(128, dtype=f16)

    b_in, b_rec = gru_bias[0].astype(f32), gru_bias[1].astype(f32)
    bzr = (b_in + b_rec)[:1024].reshape(8, 128).T.copy()      # [128, 8]
    bh = b_in[1024:].reshape(4, 128).T.copy()                 # [128, 4]
    # brech16[p, 4j+b] = b_rec[1024 + 128 j + p], replicated along t
    brech16 = np.zeros((128, 16), f16)
    for j in range(4):
        for bb in range(4):
            brech16[:, 4 * j + bb] = b_rec[1024 + 128 * j:1024 + 128 * (j + 1)]
    brech_rep = np.ascontiguousarray(
        np.repeat(brech16[:, :, None], T, axis=2))
    bu = (w1_b + w2_b).astype(f32).reshape(128, 1)

    data16 = data.astype(f16)
    h016 = h0.astype(f16)

    per_core = []
    for c in range(NC):
        sl = slice(BL * c, BL * (c + 1))
        # rows (b, i, l): t = 32 i + 4 c + l
        bidx = np.repeat(np.arange(B), 16)
        tidx = (32 * np.tile(np.repeat(np.arange(4), 4), B)
                + 4 * c + np.tile(np.arange(4), B * 4))
        dataout = data[bidx, tidx, :].astype(f32).reshape(4, 128, N)
        per_core.append({
            "data16": data16[sl], "dataout": dataout, "h016": h016[sl],
            "R_l": R_l, "K_l": K_l, "w1_l": w1_l, "w2_l": w2_l,
            "vbuf": vbuf, "ident": ident, "bias_zr": bzr, "bias_h": bh,
            "brech_rep": brech_rep, "bias_u": bu,
        })
    return per_core


def kernel(**inputs):
    from concourse.bass_utils import run_bass_kernel_spmd

    if "nc" not in _CACHE:
        _CACHE["nc"] = _build()
    nc = _CACHE["nc"]

    args = {k: np.asarray(v) for k, v in inputs.items()}
    per_core = _prep_inputs(
        args["data"], args["h0"], args["gru_kernel"], args["gru_rkernel"],
        args["gru_bias"], args["w1_w"], args["w1_b"], args["w2_w"],
        args["w2_b"], args["v_w"], args["v_b"])

    if "warm" not in _CACHE:
        # first execution after NEFF load can race; discard it
        run_bass_kernel_spmd(nc, per_core, core_ids=list(range(NC)))
        _CACHE["warm"] = True
    res = run_bass_kernel_spmd(nc, per_core, core_ids=list(range(NC)))
    _CACHE["last_res"] = res

    out = np.empty((B, T, N), np.float32)
    for c in range(NC):
        o = res.results[c]["out"].reshape(512, N)
        bidx = np.repeat(np.arange(B), 16)
        tidx = (32 * np.tile(np.repeat(np.arange(4), 4), B)
                + 4 * c + np.tile(np.arange(4), B * 4))
        out[bidx, tidx, :] = o
    return out


# revision 25
# speedup vs baseline: 1.1231x; 1.1231x over previous
"""Trainium2 Bass kernel for nn_Encoder (GRU + input attention).

Shapes (hardcoded): B=32, T=128, N=256, H=512; 8 NeuronCores, batch
sharded 4 examples/core.

Math (matching the reference):
  hs = GRU scan over T steps (Keras GRUCell, reset_after=True, gates z,r,h)
  score_x[b,n,u] = sum_t data[b,t,n] w1_w[t,u] + w1_b[u]
  hp[t,b,u]     = hs[t,b,:] @ w2_w + w2_b[u]
  score[t,b,n]  = sum_u v[u] tanh(score_x[b,n,u] + hp[t,b,u])   (+v_b: softmax-invariant)
  alpha = softmax_n(score);  out[b,t,:] = data[b,t,:] * alpha[(b*T+t)//B, (b*T+t)%B, :]

Per-core layout: u (or H-chunks) on partitions. The recurrent matmul keeps
R chunks stationary (fp16, FWL) and streams h^T (128,4) slices; gate adds for
z/r are folded into the PSUM accumulation via an identity-matmul; sigmoid is
computed as (1+tanh(x/2))/2 so tanh+exp live in one activation table set.
"""

import numpy as np

B, T, N, H = 32, 128, 256, 512
NC = 8           # cores
BL = B // NC     # batch per core (4)
H3 = 3 * H

_CACHE = {}
DEBUG = False


def _build():
    import concourse.bass as bass
    import concourse.bacc as bacc
    import concourse.tile as tile
    import concourse.mybir as mybir

    f16 = mybir.dt.float16
    f32 = mybir.dt.float32
    Alu = mybir.AluOpType
    Act = mybir.ActivationFunctionType

    nc = bacc.Bacc("TRN2", target_bir_lowering=False, debug=False)

    # ---- dram I/O ----
    d_data16 = nc.dram_tensor("data16", [BL, T, N], f16, kind="ExternalInput")
    d_dataout = nc.dram_tensor("dataout", [4, 128, N], f32, kind="ExternalInput")
    d_h016 = nc.dram_tensor("h016", [BL, H], f16, kind="ExternalInput")
    f8 = mybir.dt.float8e4
    d_R8 = nc.dram_tensor("R8_l", [128, 4, 8, 128], f8, kind="ExternalInput")
    d_Rh = nc.dram_tensor("Rh_l", [128, 4, 4, 128], f16, kind="ExternalInput")
    d_K = nc.dram_tensor("K_l", [128, 2, 12, 128], f16, kind="ExternalInput")
    d_w1 = nc.dram_tensor("w1_l", [128, 128], f16, kind="ExternalInput")
    d_w2 = nc.dram_tensor("w2_l", [128, 4, 128], f16, kind="ExternalInput")
    d_vbuf = nc.dram_tensor("vbuf", [128, 257], f16, kind="ExternalInput")
    d_ident = nc.dram_tensor("ident", [128, 128], f16, kind="ExternalInput")
    d_bzr = nc.dram_tensor("bias_zr", [128, 8], f32, kind="ExternalInput")
    d_bh = nc.dram_tensor("bias_h", [128, 4], f32, kind="ExternalInput")
    d_brech = nc.dram_tensor("brech_rep", [128, 16, T], f16, kind="ExternalInput")
    d_bu = nc.dram_tensor("bias_u", [128, 1], f32, kind="ExternalInput")
    d_out = nc.dram_tensor("out", [4, 128, N], f32, kind="ExternalOutput")
    if DEBUG:
        d_hs = nc.dram_tensor("hs_dump", [128, T + 1, 16], f16,
                              kind="ExternalOutput")
        d_sxd = nc.dram_tensor("sx_dump", [128, BL, N], f16,
                               kind="ExternalOutput")
        d_alp = nc.dram_tensor("alpha_dump", [4, 128, N], f16,
                               kind="ExternalOutput")
        d_amx = nc.dram_tensor("addmx_dump", [128, 48, T], f16,
                               kind="ExternalOutput")
        d_xhd = nc.dram_tensor("xh_dump", [128, 16, T], f16,
                               kind="ExternalOutput")

    LAG_E = 8    # e-tile tanh lag behind the scan
    LAG_S = 12   # score/softmax lag (slack lets the scheduler fill ACT gaps)

    with tile.TileContext(nc) as tc:
        with (
            tc.tile_pool(name="const", bufs=1) as cpool,
            tc.tile_pool(name="work", bufs=3) as wpool,
            tc.tile_pool(name="ebuf", bufs=3) as epool,
            tc.tile_pool(name="hpbuf", bufs=2) as hppool,
            tc.tile_pool(name="mh", bufs=2, space="PSUM") as mhpool,
            tc.tile_pool(name="bigps", bufs=2, space="PSUM") as bpool,
            tc.tile_pool(name="hpps", bufs=2, space="PSUM") as hpspool,
        ):
            # ---- persistent tiles ----
            t_R8 = cpool.tile([128, 4, 8, 128], f8)
            t_Rh = cpool.tile([128, 4, 4, 128], f16)
            t_K = cpool.tile([128, 2, 12, 128], f16)
            t_w1 = cpool.tile([128, 128], f16)
            t_w2 = cpool.tile([128, 4, 128], f16)
            t_vbuf = cpool.tile([128, 257], f16)
            t_ident = cpool.tile([128, 128], f16)
            t_bzr = cpool.tile([128, 8], f32)
            t_bh = cpool.tile([128, 4], f32)
            t_bu = cpool.tile([128, 1], f32)
            t_d16 = [cpool.tile([128, N], f16, tag=f"d16_{b}", name=f"d16_{b}")
                     for b in range(BL)]
            t_dT = cpool.tile([128, 2, BL, 128], f16)      # dataT [p, nc, b, t]
            t_h0 = cpool.tile([BL, H], f16)
            t_addmx = cpool.tile([128, 48, T], f16)        # [mx_zr' | b_rec_h] per t
            t_xh = cpool.tile([128, 16, T], f16)           # xh' per t
            t_sx = cpool.tile([128, BL, N], f16)           # score_x' per b
            t_hs = cpool.tile([128, T + 1, 16], f16)       # h^T packed, slot t+1 = hs[t]
            t_alpha = [cpool.tile([128, N], f16, tag=f"alpha_{k}", name=f"alpha_{k}")
                       for k in range(4)]
            t_ssum = cpool.tile([128, 1], f32)
            t_rinv = cpool.tile([128, 1], f32)

            # ---- DMA in ----
            for b in range(BL):
                nc.sync.dma_start(out=t_d16[b][:, :], in_=d_data16.ap()[b, :, :])
            nc.sync.dma_start(out=t_R8[:, :, :, :], in_=d_R8.ap()[:, :, :, :])
            nc.sync.dma_start(out=t_Rh[:, :, :, :], in_=d_Rh.ap()[:, :, :, :])
            nc.sync.dma_start(out=t_K[:, :, :, :], in_=d_K.ap()[:, :, :, :])
            nc.sync.dma_start(out=t_w1[:, :], in_=d_w1.ap()[:, :])
            nc.sync.dma_start(out=t_w2[:, :, :], in_=d_w2.ap()[:, :, :])
            nc.sync.dma_start(out=t_vbuf[:, :], in_=d_vbuf.ap()[:, :])
            nc.sync.dma_start(out=t_ident[:, :], in_=d_ident.ap()[:, :])
            nc.sync.dma_start(out=t_bzr[:, :], in_=d_bzr.ap()[:, :])
            nc.sync.dma_start(out=t_bh[:, :], in_=d_bh.ap()[:, :])
            nc.sync.dma_start(out=t_addmx[:, 32:48, :], in_=d_brech.ap()[:, :, :])
            nc.sync.dma_start(out=t_bu[:, :], in_=d_bu.ap()[:, :])
            nc.sync.dma_start(out=t_h0[:, :], in_=d_h016.ap()[:, :])

            # ---- prologue: h0^T into hs slot 0 ----
            for j in range(4):
                ps = bpool.tile([128, 128], f16, tag="bigps")
                nc.tensor.transpose(ps[:, 0:BL], t_h0[0:BL, 128 * j:128 * (j + 1)],
                                    t_ident[0:BL, 0:BL])
                nc.vector.tensor_copy(t_hs[:, 0, 4 * j:4 * j + 4], ps[:, 0:BL])

            # ---- prologue: data^T  [p, nc, b, t] ----
            for b in range(BL):
                for n2 in range(2):
                    ps = bpool.tile([128, 128], f16, tag="bigps")
                    nc.tensor.transpose(ps[:, :], t_d16[b][:, 128 * n2:128 * (n2 + 1)],
                                        t_ident[:, :])
                    nc.vector.tensor_copy(t_dT[:, n2, b, :], ps[:, :])

            # ---- prologue: mx = data @ K (+biases), scattered per t ----
            for uc in range(12):
                ps = bpool.tile([128, BL, 128], f32, tag="bigps")
                for n2 in range(2):
                    nc.tensor.matmul(ps[:, :, :], t_K[:, n2, uc, :],
                                     t_dT[:, n2, :, :],
                                     start=(n2 == 0), stop=(n2 == 1))
                g, j = divmod(uc, 4)
                # psum free order is (b, t); dest free dims (b-within-col, t)
                if g < 2:
                    nc.scalar.activation(
                        t_addmx[:, 4 * uc:4 * uc + 4, :], ps[:, :, :],
                        Act.Identity, bias=t_bzr[:, uc:uc + 1])
                else:
                    nc.scalar.activation(
                        t_xh[:, 4 * j:4 * j + 4, :], ps[:, :, :],
                        Act.Identity, bias=t_bh[:, j:j + 1])

            # ---- prologue: score_x' ----
            for b in range(BL):
                ps = bpool.tile([128, N], f32, tag="bigps")
                nc.tensor.matmul(ps[:, :], t_w1[:, :], t_d16[b][:, :],
                                 start=True, stop=True)
                nc.scalar.activation(t_sx[:, b, :], ps[:, :],
                                     Act.Identity, bias=t_bu[:, :])

            # ---- helpers for the lagged attention pipeline ----
            hp_tiles = {}  # block index -> sbuf tile [128, 32] f32

            def emit_hp_block(blk):
                t0 = 8 * blk
                ps = hpspool.tile([128, 32], f32)
                for hc in range(4):
                    nc.tensor.matmul(ps[:, :], t_w2[:, hc, :],
                                     t_hs[:, t0 + 1:t0 + 9, 4 * hc:4 * hc + 4],
                                     start=(hc == 0), stop=(hc == 3))
                hp = hppool.tile([128, 32], f32)
                nc.vector.tensor_copy(hp[:, :], ps[:, :])
                hp_tiles[blk] = hp

            e_tiles = {}  # ta -> e tile [128, BL, N] f16

            def emit_e(ta):
                hp = hp_tiles[ta // 8]
                tl = ta % 8
                ei = epool.tile([128, BL, N], f16, tag="ein")
                for b in range(BL):
                    nc.vector.tensor_scalar_add(ei[:, b, :], t_sx[:, b, :],
                                                hp[:, 4 * tl + b:4 * tl + b + 1])
                et = epool.tile([128, BL, N], f16, tag="etile", bufs=6)
                nc.scalar.activation(et[:, 0:2, :], ei[:, 0:2, :], Act.Tanh)
                nc.scalar.activation(et[:, 2:4, :], ei[:, 2:4, :], Act.Tanh)
                e_tiles[ta] = et

            score_ps = {}  # group (8 steps) -> psum tile [128, N] f32

            def emit_score_mm(ta):
                et = e_tiles.pop(ta)
                g, tl = divmod(ta, 8)
                if tl == 0:
                    score_ps[g] = bpool.tile([128, N], f32, tag="bigps",
                                             name=f"scps_{g}")
                ps = score_ps[g]
                q = (ta % 32) // 8          # 32-aligned row group within psum
                for l in range(BL):
                    c = 4 * tl + l          # column within the 32-wide window
                    nc.tensor.matmul(ps[32 * q:32 * q + 32, :],
                                     t_vbuf[:, 128 - c:160 - c], et[:, l, :],
                                     start=(tl == 0 and l == 0),
                                     stop=(tl == 7 and l == BL - 1),
                                     skip_group_check=True,
                                     tile_position=(0, 32 * q))

            def emit_softmax_group(g):
                ps = score_ps.pop(g)
                p0 = 32 * (g % 4)
                ex = wpool.tile([128, N], f16, tag="expv")
                nc.scalar.activation(ex[p0:p0 + 32, :], ps[p0:p0 + 32, :], Act.Exp,
                                     accum_out=t_ssum[p0:p0 + 32, :])
                nc.vector.reciprocal(t_rinv[p0:p0 + 32, :], t_ssum[p0:p0 + 32, :])
                nc.vector.tensor_scalar(
                    t_alpha[g // 4][p0:p0 + 32, :], ex[p0:p0 + 32, :],
                    t_rinv[p0:p0 + 32, :], None, Alu.mult)

            # ---- main scan loop ----
            for t in range(T):
                if t % 8 == 0 and t >= 8:
                    emit_hp_block(t // 8 - 1)

                mh = mhpool.tile([128, 48], f32)
                # bias/mx seed: writes all 48 cols, start=True clears the bank
                nc.tensor.matmul(mh[:, 0:48], t_ident[:, :], t_addmx[:, :, t],
                                 start=True, stop=False, skip_group_check=True)
                # z/r chunks first, then h chunks — all accumulate
                for uc in range(12):
                    for kc in range(4):
                        lhsT = (t_R8[:, kc, uc, :] if uc < 8
                                else t_Rh[:, kc, uc - 8, :])
                        nc.tensor.matmul(mh[:, 4 * uc:4 * uc + 4], lhsT,
                                         t_hs[:, t, 4 * kc:4 * kc + 4],
                                         start=False,
                                         stop=(uc == 11 and kc == 3),
                                         skip_group_check=True)

                # gates
                tzr = wpool.tile([128, 32], f16, tag="tzr")
                zr = wpool.tile([128, 32], f16, tag="zr")
                t2 = wpool.tile([128, 16], f16, tag="t2")
                t3 = wpool.tile([128, 16], f16, tag="t3")
                cg = wpool.tile([128, 16], f16, tag="cg")
                dg = wpool.tile([128, 16], f16, tag="dg")
                mg = wpool.tile([128, 16], f16, tag="mg")

                nc.scalar.activation(tzr[:, :], mh[:, 0:32], Act.Tanh, scale=0.5)
                nc.vector.tensor_scalar(zr[:, :], tzr[:, :], 1.0, 0.5,
                                        Alu.add, Alu.mult)
                zg = zr[:, 0:16]
                rg = zr[:, 16:32]
                nc.vector.tensor_tensor(t2[:, :], rg[:, :], mh[:, 32:48], Alu.mult)
                nc.vector.tensor_tensor(t3[:, :], t2[:, :], t_xh[:, :, t], Alu.add)
                nc.scalar.activation(cg[:, :], t3[:, :], Act.Tanh)
                nc.vector.tensor_tensor(dg[:, :], t_hs[:, t, :], cg[:, :],
                                        Alu.subtract)
                nc.vector.tensor_tensor(mg[:, :], zg[:, :], dg[:, :], Alu.mult)
                nc.vector.tensor_tensor(t_hs[:, t + 1, :], cg[:, :], mg[:, :],
                                        Alu.add)

                # lagged attention
                if t >= LAG_E:
                    emit_e(t - LAG_E)
                if t >= LAG_S:
                    emit_score_mm(t - LAG_S)
                    if (t - LAG_S) % 8 == 7:
                        emit_softmax_group((t - LAG_S) // 8)

            # ---- attention epilogue ----
            emit_hp_block(15)
            for ta in range(T - LAG_E, T):
                emit_e(ta)
            for ta in range(T - LAG_S, T):
                emit_score_mm(ta)
                if ta % 8 == 7:
                    emit_softmax_group(ta // 8)

            if DEBUG:
                nc.sync.dma_start(out=d_hs.ap()[:, :, :], in_=t_hs[:, :, :])
                nc.sync.dma_start(out=d_sxd.ap()[:, :, :], in_=t_sx[:, :, :])
                for k in range(4):
                    nc.sync.dma_start(out=d_alp.ap()[k, :, :],
                                      in_=t_alpha[k][:, :])
                nc.sync.dma_start(out=d_amx.ap()[:, :, :], in_=t_addmx[:, :, :])
                nc.sync.dma_start(out=d_xhd.ap()[:, :, :], in_=t_xh[:, :, :])

            # ---- final out = data * alpha ----
            for k in range(4):
                dt_ = wpool.tile([128, N], f32, tag="dmul")
                ot = wpool.tile([128, N], f32, tag="omul")
                nc.sync.dma_start(out=dt_[:, :], in_=d_dataout.ap()[k, :, :])
                nc.vector.tensor_tensor(ot[:, :], dt_[:, :], t_alpha[k][:, :],
                                        Alu.mult)
                nc.sync.dma_start(out=d_out.ap()[k, :, :], in_=ot[:, :])

    nc.compile()
    return nc


def _prep_inputs(data, h0, gru_kernel, gru_rkernel, gru_bias,
                 w1_w, w1_b, w2_w, w2_b, v_w, v_b):
    f16 = np.float16
    f32 = np.float32

    import ml_dtypes
    R_all = np.ascontiguousarray(
        gru_rkernel.reshape(4, 128, 12, 128).transpose(1, 0, 2, 3))
    R8_l = R_all[:, :, 0:8, :].astype(ml_dtypes.float8_e4m3)
    Rh_l = R_all[:, :, 8:12, :].astype(f16)
    K16 = gru_kernel.astype(f16)                      # (256, 1536)
    K_l = np.ascontiguousarray(
        K16.reshape(2, 128, 12, 128).transpose(1, 0, 2, 3))
    w1_l = w1_w.astype(f16)                           # (128, 128)
    w2_l = np.ascontiguousarray(
        w2_w.astype(f16).reshape(4, 128, 128).transpose(1, 0, 2))
    vbuf = np.zeros((128, 257), f16)
    vbuf[:, 128] = v_w[:, 0].astype(f16)
    ident = np.eye(128, dtype=f16)

    b_in, b_rec = gru_bias[0].astype(f32), gru_bias[1].astype(f32)
    bzr = (b_in + b_rec)[:1024].reshape(8, 128).T.copy()      # [128, 8]
    bh = b_in[1024:].reshape(4, 128).T.copy()                 # [128, 4]
    # brech16[p, 4j+b] = b_rec[1024 + 128 j + p], replicated along t
    brech16 = np.zeros((128, 16), f16)
    for j in range(4):
        for bb in range(4):
            brech16[:, 4 * j + bb] = b_rec[1024 + 128 * j:1024 + 128 * (j + 1)]
    brech_rep = np.ascontiguousarray(
        np.repeat(brech16[:, :, None], T, axis=2))
    bu = (w1_b + w2_b).astype(f32).reshape(128, 1)

    data16 = data.astype(f16)
    h016 = h0.astype(f16)

    per_core = []
    for c in range(NC):
        sl = slice(BL * c, BL * (c + 1))
        # rows (b, i, l): t = 32 i + 4 c + l
        bidx = np.repeat(np.arange(B), 16)
        tidx = (32 * np.tile(np.repeat(np.arange(4), 4), B)
                + 4 * c + np.tile(np.arange(4), B * 4))
        dataout = data[bidx, tidx, :].astype(f32).reshape(4, 128, N)
        per_core.append({
            "data16": data16[sl], "dataout": dataout, "h016": h016[sl],
            "R8_l": R8_l, "Rh_l": Rh_l, "K_l": K_l, "w1_l": w1_l, "w2_l": w2_l,
            "vbuf": vbuf, "ident": ident, "bias_zr": bzr, "bias_h": bh,
            "brech_rep": brech_rep, "bias_u": bu,
        })
    return per_core


def kernel(**inputs):
    from concourse.bass_utils import run_bass_kernel_spmd

    if "nc" not in _CACHE:
        _CACHE["nc"] = _build()
    nc = _CACHE["nc"]

    args = {k: np.asarray(v) for k, v in inputs.items()}
    per_core = _prep_inputs(
        args["data"], args["h0"], args["gru_kernel"], args["gru_rkernel"],
        args["gru_bias"], args["w1_w"], args["w1_b"], args["w2_w"],
        args["w2_b"], args["v_w"], args["v_b"])

    if "warm" not in _CACHE:
        # first execution after NEFF load can race; discard it
        run_bass_kernel_spmd(nc, per_core, core_ids=list(range(NC)))
        _CACHE["warm"] = True
    res = run_bass_kernel_spmd(nc, per_core, core_ids=list(range(NC)))
    _CACHE["last_res"] = res

    out = np.empty((B, T, N), np.float32)
    for c in range(NC):
        o = res.results[c]["out"].reshape(512, N)
        bidx = np.repeat(np.arange(B), 16)
        tidx = (32 * np.tile(np.repeat(np.arange(4), 4), B)
                + 4 * c + np.tile(np.arange(4), B * 4))
        out[bidx, tidx, :] = o
    return out


# revision 27
# speedup vs baseline: 1.1311x; 1.0071x over previous
"""Trainium2 Bass kernel for nn_Encoder (GRU + input attention).

Shapes (hardcoded): B=32, T=128, N=256, H=512; 8 NeuronCores, batch
sharded 4 examples/core.

Math (matching the reference):
  hs = GRU scan over T steps (Keras GRUCell, reset_after=True, gates z,r,h)
  score_x[b,n,u] = sum_t data[b,t,n] w1_w[t,u] + w1_b[u]
  hp[t,b,u]     = hs[t,b,:] @ w2_w + w2_b[u]
  score[t,b,n]  = sum_u v[u] tanh(score_x[b,n,u] + hp[t,b,u])   (+v_b: softmax-invariant)
  alpha = softmax_n(score);  out[b,t,:] = data[b,t,:] * alpha[(b*T+t)//B, (b*T+t)%B, :]

Per-core layout: u (or H-chunks) on partitions. The recurrent matmul keeps
R chunks stationary (fp16, FWL) and streams h^T (128,4) slices; gate adds for
z/r are folded into the PSUM accumulation via an identity-matmul; sigmoid is
computed as (1+tanh(x/2))/2 so tanh+exp live in one activation table set.
"""

import numpy as np

B, T, N, H = 32, 128, 256, 512
NC = 8           # cores
BL = B // NC     # batch per core (4)
H3 = 3 * H

_CACHE = {}
DEBUG = False


def _build():
    import concourse.bass as bass
    import concourse.bacc as bacc
    import concourse.tile as tile
    import concourse.mybir as mybir

    f16 = mybir.dt.float16
    f32 = mybir.dt.float32
    Alu = mybir.AluOpType
    Act = mybir.ActivationFunctionType

    nc = bacc.Bacc("TRN2", target_bir_lowering=False, debug=False)

    # ---- dram I/O ----
    d_data16 = nc.dram_tensor("data16", [BL, T, N], f16, kind="ExternalInput")
    d_dataout = nc.dram_tensor("dataout", [4, 128, N], f32, kind="ExternalInput")
    d_h016 = nc.dram_tensor("h016", [BL, H], f16, kind="ExternalInput")
    f8 = mybir.dt.float8e4
    d_R8 = nc.dram_tensor("R8_l", [128, 4, 8, 128], f8, kind="ExternalInput")
    d_Rh = nc.dram_tensor("Rh_l", [128, 4, 4, 128], f16, kind="ExternalInput")
    d_K = nc.dram_tensor("K_l", [128, 2, 12, 128], f16, kind="ExternalInput")
    d_w1 = nc.dram_tensor("w1_l", [128, 128], f16, kind="ExternalInput")
    d_w2 = nc.dram_tensor("w2_l", [128, 4, 128], f16, kind="ExternalInput")
    d_vbuf = nc.dram_tensor("vbuf", [128, 257], f16, kind="ExternalInput")
    d_ident = nc.dram_tensor("ident", [128, 128], f16, kind="ExternalInput")
    d_bzr = nc.dram_tensor("bias_zr", [128, 8], f32, kind="ExternalInput")
    d_bh = nc.dram_tensor("bias_h", [128, 4], f32, kind="ExternalInput")
    d_brech = nc.dram_tensor("brech_rep", [128, 16, T], f16, kind="ExternalInput")
    d_bu = nc.dram_tensor("bias_u", [128, 1], f32, kind="ExternalInput")
    d_out = nc.dram_tensor("out", [4, 128, N], f32, kind="ExternalOutput")
    if DEBUG:
        d_hs = nc.dram_tensor("hs_dump", [128, T + 1, 16], f16,
                              kind="ExternalOutput")
        d_sxd = nc.dram_tensor("sx_dump", [128, BL, N], f16,
                               kind="ExternalOutput")
        d_alp = nc.dram_tensor("alpha_dump", [4, 128, N], f16,
                               kind="ExternalOutput")
        d_amx = nc.dram_tensor("addmx_dump", [128, 48, T], f16,
                               kind="ExternalOutput")
        d_xhd = nc.dram_tensor("xh_dump", [128, 16, T], f16,
                               kind="ExternalOutput")

    LAG_E = 8    # e-tile tanh lag behind the scan
    LAG_S = 12   # score/softmax lag (slack lets the scheduler fill ACT gaps)

    with tile.TileContext(nc) as tc:
        with (
            tc.tile_pool(name="const", bufs=1) as cpool,
            tc.tile_pool(name="work", bufs=3) as wpool,
            tc.tile_pool(name="ebuf", bufs=3) as epool,
            tc.tile_pool(name="hpbuf", bufs=2) as hppool,
            tc.tile_pool(name="mh", bufs=2, space="PSUM") as mhpool,
            tc.tile_pool(name="mhh", bufs=2, space="PSUM") as mhhpool,
            tc.tile_pool(name="bigps", bufs=2, space="PSUM") as bpool,
            tc.tile_pool(name="hpps", bufs=2, space="PSUM") as hpspool,
        ):
            # ---- persistent tiles ----
            t_R8 = cpool.tile([128, 4, 8, 128], f8)
            t_Rh = cpool.tile([128, 4, 4, 128], f16)
            t_K = cpool.tile([128, 2, 12, 128], f16)
            t_w1 = cpool.tile([128, 128], f16)
            t_w2 = cpool.tile([128, 4, 128], f16)
            t_vbuf = cpool.tile([128, 257], f16)
            t_ident = cpool.tile([128, 128], f16)
            t_bzr = cpool.tile([128, 8], f32)
            t_bh = cpool.tile([128, 4], f32)
            t_bu = cpool.tile([128, 1], f32)
            t_d16 = [cpool.tile([128, N], f16, tag=f"d16_{b}", name=f"d16_{b}")
                     for b in range(BL)]
            t_dT = cpool.tile([128, 2, BL, 128], f16)      # dataT [p, nc, b, t]
            t_h0 = cpool.tile([BL, H], f16)
            t_addmx = cpool.tile([128, 48, T], f16)        # [mx_zr' | b_rec_h] per t
            t_xh = cpool.tile([128, 16, T], f16)           # xh' per t
            t_sx = cpool.tile([128, BL, N], f16)           # score_x' per b
            t_hs = cpool.tile([128, T + 1, 16], f16)       # h^T packed, slot t+1 = hs[t]
            t_alpha = [cpool.tile([128, N], f16, tag=f"alpha_{k}", name=f"alpha_{k}")
                       for k in range(4)]
            t_ssum = cpool.tile([128, 1], f32)
            t_rinv = cpool.tile([128, 1], f32)

            # ---- DMA in ----
            for b in range(BL):
                nc.sync.dma_start(out=t_d16[b][:, :], in_=d_data16.ap()[b, :, :])
            nc.sync.dma_start(out=t_R8[:, :, :, :], in_=d_R8.ap()[:, :, :, :])
            nc.sync.dma_start(out=t_Rh[:, :, :, :], in_=d_Rh.ap()[:, :, :, :])
            nc.sync.dma_start(out=t_K[:, :, :, :], in_=d_K.ap()[:, :, :, :])
            nc.sync.dma_start(out=t_w1[:, :], in_=d_w1.ap()[:, :])
            nc.sync.dma_start(out=t_w2[:, :, :], in_=d_w2.ap()[:, :, :])
            nc.sync.dma_start(out=t_vbuf[:, :], in_=d_vbuf.ap()[:, :])
            nc.sync.dma_start(out=t_ident[:, :], in_=d_ident.ap()[:, :])
            nc.sync.dma_start(out=t_bzr[:, :], in_=d_bzr.ap()[:, :])
            nc.sync.dma_start(out=t_bh[:, :], in_=d_bh.ap()[:, :])
            nc.sync.dma_start(out=t_addmx[:, 32:48, :], in_=d_brech.ap()[:, :, :])
            nc.sync.dma_start(out=t_bu[:, :], in_=d_bu.ap()[:, :])
            nc.sync.dma_start(out=t_h0[:, :], in_=d_h016.ap()[:, :])

            # ---- prologue: h0^T into hs slot 0 ----
            for j in range(4):
                ps = bpool.tile([128, 128], f16, tag="bigps")
                nc.tensor.transpose(ps[:, 0:BL], t_h0[0:BL, 128 * j:128 * (j + 1)],
                                    t_ident[0:BL, 0:BL])
                nc.vector.tensor_copy(t_hs[:, 0, 4 * j:4 * j + 4], ps[:, 0:BL])

            # ---- prologue: data^T  [p, nc, b, t] ----
            for b in range(BL):
                for n2 in range(2):
                    ps = bpool.tile([128, 128], f16, tag="bigps")
                    nc.tensor.transpose(ps[:, :], t_d16[b][:, 128 * n2:128 * (n2 + 1)],
                                        t_ident[:, :])
                    nc.vector.tensor_copy(t_dT[:, n2, b, :], ps[:, :])

            # ---- prologue: mx = data @ K (+biases), scattered per t ----
            for uc in range(12):
                ps = bpool.tile([128, BL, 128], f32, tag="bigps")
                for n2 in range(2):
                    nc.tensor.matmul(ps[:, :, :], t_K[:, n2, uc, :],
                                     t_dT[:, n2, :, :],
                                     start=(n2 == 0), stop=(n2 == 1))
                g, j = divmod(uc, 4)
                # psum free order is (b, t); dest free dims (b-within-col, t)
                if g < 2:
                    nc.scalar.activation(
                        t_addmx[:, 4 * uc:4 * uc + 4, :], ps[:, :, :],
                        Act.Identity, bias=t_bzr[:, uc:uc + 1])
                else:
                    nc.scalar.activation(
                        t_xh[:, 4 * j:4 * j + 4, :], ps[:, :, :],
                        Act.Identity, bias=t_bh[:, j:j + 1])

            # ---- prologue: score_x' ----
            for b in range(BL):
                ps = bpool.tile([128, N], f32, tag="bigps")
                nc.tensor.matmul(ps[:, :], t_w1[:, :], t_d16[b][:, :],
                                 start=True, stop=True)
                nc.scalar.activation(t_sx[:, b, :], ps[:, :],
                                     Act.Identity, bias=t_bu[:, :])

            # ---- helpers for the lagged attention pipeline ----
            hp_tiles = {}  # block index -> sbuf tile [128, 32] f32

            def emit_hp_block(blk):
                t0 = 8 * blk
                ps = hpspool.tile([128, 32], f32)
                for hc in range(4):
                    nc.tensor.matmul(ps[:, :], t_w2[:, hc, :],
                                     t_hs[:, t0 + 1:t0 + 9, 4 * hc:4 * hc + 4],
                                     start=(hc == 0), stop=(hc == 3))
                hp = hppool.tile([128, 32], f32)
                nc.vector.tensor_copy(hp[:, :], ps[:, :])
                hp_tiles[blk] = hp

            e_tiles = {}  # ta -> e tile [128, BL, N] f16

            def emit_e(ta):
                hp = hp_tiles[ta // 8]
                tl = ta % 8
                ei = epool.tile([128, BL, N], f16, tag="ein")
                for b in range(BL):
                    nc.vector.tensor_scalar_add(ei[:, b, :], t_sx[:, b, :],
                                                hp[:, 4 * tl + b:4 * tl + b + 1])
                et = epool.tile([128, BL, N], f16, tag="etile", bufs=6)
                nc.scalar.activation(et[:, 0:2, :], ei[:, 0:2, :], Act.Tanh)
                nc.scalar.activation(et[:, 2:4, :], ei[:, 2:4, :], Act.Tanh)
                e_tiles[ta] = et

            score_ps = {}  # group (8 steps) -> psum tile [128, N] f32

            def emit_score_mm(ta):
                et = e_tiles.pop(ta)
                g, tl = divmod(ta, 8)
                if tl == 0:
                    score_ps[g] = bpool.tile([128, N], f32, tag="bigps",
                                             name=f"scps_{g}")
                ps = score_ps[g]
                q = (ta % 32) // 8          # 32-aligned row group within psum
                for l in range(BL):
                    c = 4 * tl + l          # column within the 32-wide window
                    nc.tensor.matmul(ps[32 * q:32 * q + 32, :],
                                     t_vbuf[:, 128 - c:160 - c], et[:, l, :],
                                     start=(tl == 0 and l == 0),
                                     stop=(tl == 7 and l == BL - 1),
                                     skip_group_check=True,
                                     tile_position=(0, 32 * q))

            def emit_softmax_group(g):
                ps = score_ps.pop(g)
                p0 = 32 * (g % 4)
                ex = wpool.tile([128, N], f16, tag="expv")
                nc.scalar.activation(ex[p0:p0 + 32, :], ps[p0:p0 + 32, :], Act.Exp,
                                     accum_out=t_ssum[p0:p0 + 32, :])
                nc.vector.reciprocal(t_rinv[p0:p0 + 32, :], t_ssum[p0:p0 + 32, :])
                nc.vector.tensor_scalar(
                    t_alpha[g // 4][p0:p0 + 32, :], ex[p0:p0 + 32, :],
                    t_rinv[p0:p0 + 32, :], None, Alu.mult)

            # ---- main scan loop ----
            for t in range(T):
                if t % 8 == 0 and t >= 8:
                    emit_hp_block(t // 8 - 1)

                mh = mhpool.tile([128, 32], f32)
                mhh = mhhpool.tile([128, 16], f32)
                # bias/mx seeds: start=True clears each bank
                nc.tensor.matmul(mh[:, 0:32], t_ident[:, :], t_addmx[:, 0:32, t],
                                 start=True, stop=False, skip_group_check=True)
                for uc in range(8):
                    for kc in range(4):
                        nc.tensor.matmul(mh[:, 4 * uc:4 * uc + 4],
                                         t_R8[:, kc, uc, :],
                                         t_hs[:, t, 4 * kc:4 * kc + 4],
                                         start=False,
                                         stop=(uc == 7 and kc == 3),
                                         skip_group_check=True)
                nc.tensor.matmul(mhh[:, 0:16], t_ident[:, :], t_addmx[:, 32:48, t],
                                 start=True, stop=False, skip_group_check=True)
                for uc in range(4):
                    for kc in range(4):
                        nc.tensor.matmul(mhh[:, 4 * uc:4 * uc + 4],
                                         t_Rh[:, kc, uc, :],
                                         t_hs[:, t, 4 * kc:4 * kc + 4],
                                         start=False,
                                         stop=(uc == 3 and kc == 3),
                                         skip_group_check=True)

                # gates
                tzr = wpool.tile([128, 32], f16, tag="tzr")
                zr = wpool.tile([128, 32], f16, tag="zr")
                t2 = wpool.tile([128, 16], f16, tag="t2")
                t3 = wpool.tile([128, 16], f16, tag="t3")
                cg = wpool.tile([128, 16], f16, tag="cg")
                dg = wpool.tile([128, 16], f16, tag="dg")
                mg = wpool.tile([128, 16], f16, tag="mg")

                nc.scalar.activation(tzr[:, :], mh[:, 0:32], Act.Tanh, scale=0.5)
                nc.vector.tensor_scalar(zr[:, :], tzr[:, :], 1.0, 0.5,
                                        Alu.add, Alu.mult)
                zg = zr[:, 0:16]
                rg = zr[:, 16:32]
                nc.vector.tensor_tensor(t2[:, :], rg[:, :], mhh[:, 0:16], Alu.mult)
                nc.vector.tensor_tensor(t3[:, :], t2[:, :], t_xh[:, :, t], Alu.add)
                nc.scalar.activation(cg[:, :], t3[:, :], Act.Tanh)
                nc.vector.tensor_tensor(dg[:, :], t_hs[:, t, :], cg[:, :],
                                        Alu.subtract)
                nc.vector.tensor_tensor(mg[:, :], zg[:, :], dg[:, :], Alu.mult)
                nc.vector.tensor_tensor(t_hs[:, t + 1, :], cg[:, :], mg[:, :],
                                        Alu.add)

                # lagged attention
                if t >= LAG_E:
                    emit_e(t - LAG_E)
                if t >= LAG_S:
                    emit_score_mm(t - LAG_S)
                    if (t - LAG_S) % 8 == 7:
                        emit_softmax_group((t - LAG_S) // 8)

            # ---- attention epilogue ----
            emit_hp_block(15)
            for ta in range(T - LAG_E, T):
                emit_e(ta)
            for ta in range(T - LAG_S, T):
                emit_score_mm(ta)
                if ta % 8 == 7:
                    emit_softmax_group(ta // 8)

            if DEBUG:
                nc.sync.dma_start(out=d_hs.ap()[:, :, :], in_=t_hs[:, :, :])
                nc.sync.dma_start(out=d_sxd.ap()[:, :, :], in_=t_sx[:, :, :])
                for k in range(4):
                    nc.sync.dma_start(out=d_alp.ap()[k, :, :],
                                      in_=t_alpha[k][:, :])
                nc.sync.dma_start(out=d_amx.ap()[:, :, :], in_=t_addmx[:, :, :])
                nc.sync.dma_start(out=d_xhd.ap()[:, :, :], in_=t_xh[:, :, :])

            # ---- final out = data * alpha ----
            for k in range(4):
                dt_ = wpool.tile([128, N], f32, tag="dmul")
                ot = wpool.tile([128, N], f32, tag="omul")
                nc.sync.dma_start(out=dt_[:, :], in_=d_dataout.ap()[k, :, :])
                nc.vector.tensor_tensor(ot[:, :], dt_[:, :], t_alpha[k][:, :],
                                        Alu.mult)
                nc.sync.dma_start(out=d_out.ap()[k, :, :], in_=ot[:, :])

    nc.compile()
    return nc


def _prep_inputs(data, h0, gru_kernel, gru_rkernel, gru_bias,
                 w1_w, w1_b, w2_w, w2_b, v_w, v_b):
    f16 = np.float16
    f32 = np.float32

    import ml_dtypes
    R_all = np.ascontiguousarray(
        gru_rkernel.reshape(4, 128, 12, 128).transpose(1, 0, 2, 3))
    R8_l = R_all[:, :, 0:8, :].astype(ml_dtypes.float8_e4m3)
    Rh_l = R_all[:, :, 8:12, :].astype(f16)
    K16 = gru_kernel.astype(f16)                      # (256, 1536)
    K_l = np.ascontiguousarray(
        K16.reshape(2, 128, 12, 128).transpose(1, 0, 2, 3))
    w1_l = w1_w.astype(f16)                           # (128, 128)
    w2_l = np.ascontiguousarray(
        w2_w.astype(f16).reshape(4, 128, 128).transpose(1, 0, 2))
    vbuf = np.zeros((128, 257), f16)
    vbuf[:, 128] = v_w[:, 0].astype(f16)
    ident = np.eye(128, dtype=f16)

    b_in, b_rec = gru_bias[0].astype(f32), gru_bias[1].astype(f32)
    bzr = (b_in + b_rec)[:1024].reshape(8, 128).T.copy()      # [128, 8]
    bh = b_in[1024:].reshape(4, 128).T.copy()                 # [128, 4]
    # brech16[p, 4j+b] = b_rec[1024 + 128 j + p], replicated along t
    brech16 = np.zeros((128, 16), f16)
    for j in range(4):
        for bb in range(4):
            brech16[:, 4 * j + bb] = b_rec[1024 + 128 * j:1024 + 128 * (j + 1)]
    brech_rep = np.ascontiguousarray(
        np.repeat(brech16[:, :, None], T, axis=2))
    bu = (w1_b + w2_b).astype(f32).reshape(128, 1)

    data16 = data.astype(f16)
    h016 = h0.astype(f16)

    per_core = []
    for c in range(NC):
        sl = slice(BL * c, BL * (c + 1))
        # rows (b, i, l): t = 32 i + 4 c + l
        bidx = np.repeat(np.arange(B), 16)
        tidx = (32 * np.tile(np.repeat(np.arange(4), 4), B)
                + 4 * c + np.tile(np.arange(4), B * 4))
        dataout = data[bidx, tidx, :].astype(f32).reshape(4, 128, N)
        per_core.append({
            "data16": data16[sl], "dataout": dataout, "h016": h016[sl],
            "R8_l": R8_l, "Rh_l": Rh_l, "K_l": K_l, "w1_l": w1_l, "w2_l": w2_l,
            "vbuf": vbuf, "ident": ident, "bias_zr": bzr, "bias_h": bh,
            "brech_rep": brech_rep, "bias_u": bu,
        })
    return per_core


def kernel(**inputs):
    from concourse.bass_utils import run_bass_kernel_spmd

    if "nc" not in _CACHE:
        _CACHE["nc"] = _build()
    nc = _CACHE["nc"]

    args = {k: np.asarray(v) for k, v in inputs.items()}
    per_core = _prep_inputs(
        args["data"], args["h0"], args["gru_kernel"], args["gru_rkernel"],
        args["gru_bias"], args["w1_w"], args["w1_b"], args["w2_w"],
        args["w2_b"], args["v_w"], args["v_b"])

    if "warm" not in _CACHE:
        # first execution after NEFF load can race; discard it
        run_bass_kernel_spmd(nc, per_core, core_ids=list(range(NC)))
        _CACHE["warm"] = True
    res = run_bass_kernel_spmd(nc, per_core, core_ids=list(range(NC)))
    _CACHE["last_res"] = res

    out = np.empty((B, T, N), np.float32)
    for c in range(NC):
        o = res.results[c]["out"].reshape(512, N)
        bidx = np.repeat(np.arange(B), 16)
        tidx = (32 * np.tile(np.repeat(np.arange(4), 4), B)
                + 4 * c + np.tile(np.arange(4), B * 4))
        out[bidx, tidx, :] = o
    return out


# revision 34
# speedup vs baseline: 1.2534x; 1.1082x over previous
"""Trainium2 Bass kernel for nn_Encoder (GRU + input attention).

Shapes (hardcoded): B=32, T=128, N=256, H=512; 8 NeuronCores, batch
sharded 4 examples/core.

Math (matching the reference):
  hs = GRU scan over T steps (Keras GRUCell, reset_after=True, gates z,r,h)
  score_x[b,n,u] = sum_t data[b,t,n] w1_w[t,u] + w1_b[u]
  hp[t,b,u]     = hs[t,b,:] @ w2_w + w2_b[u]
  score[t,b,n]  = sum_u v[u] tanh(score_x[b,n,u] + hp[t,b,u])   (+v_b: softmax-invariant)
  alpha = softmax_n(score);  out[b,t,:] = data[b,t,:] * alpha[(b*T+t)//B, (b*T+t)%B, :]

Per-core layout: u (or H-chunks) on partitions. The recurrent matmul keeps
R chunks stationary (fp16, FWL) and streams h^T (128,4) slices; gate adds for
z/r are folded into the PSUM accumulation via an identity-matmul; sigmoid is
computed as (1+tanh(x/2))/2 so tanh+exp live in one activation table set.
"""

import numpy as np

B, T, N, H = 32, 128, 256, 512
NC = 8           # cores
BL = B // NC     # batch per core (4)
H3 = 3 * H

_CACHE = {}
DEBUG = False
import os
SCAN_ONLY = os.environ.get("SCAN_ONLY", "0") == "1"


def _build():
    import concourse.bass as bass
    import concourse.bacc as bacc
    import concourse.tile as tile
    import concourse.mybir as mybir

    f16 = mybir.dt.float16
    f32 = mybir.dt.float32
    Alu = mybir.AluOpType
    Act = mybir.ActivationFunctionType

    nc = bacc.Bacc("TRN2", target_bir_lowering=False, debug=False)

    # ---- dram I/O ----
    d_data16 = nc.dram_tensor("data16", [BL, T, N], f16, kind="ExternalInput")
    d_dataout = nc.dram_tensor("dataout", [4, 128, N], f32, kind="ExternalInput")
    d_h016 = nc.dram_tensor("h016", [BL, H], f16, kind="ExternalInput")
    f8 = mybir.dt.float8e4
    d_R8 = nc.dram_tensor("R8_l", [128, 4, 8, 128], f8, kind="ExternalInput")
    d_Rh = nc.dram_tensor("Rh_l", [128, 4, 4, 128], f16, kind="ExternalInput")
    d_K = nc.dram_tensor("K_l", [128, 2, 12, 128], f16, kind="ExternalInput")
    d_w1 = nc.dram_tensor("w1_l", [128, 128], f16, kind="ExternalInput")
    d_w2 = nc.dram_tensor("w2_l", [128, 4, 128], f16, kind="ExternalInput")
    d_vbuf = nc.dram_tensor("vbuf", [128, 257], f16, kind="ExternalInput")
    d_ident = nc.dram_tensor("ident", [128, 128], f16, kind="ExternalInput")
    d_bzr = nc.dram_tensor("bias_zr", [128, 8], f32, kind="ExternalInput")
    d_bh = nc.dram_tensor("bias_h", [128, 4], f32, kind="ExternalInput")
    d_brech = nc.dram_tensor("brech_rep", [128, 16, T], f16, kind="ExternalInput")
    d_bu = nc.dram_tensor("bias_u", [128, 1], f32, kind="ExternalInput")
    d_out = nc.dram_tensor("out", [4, 128, N], f32, kind="ExternalOutput")
    if DEBUG:
        d_hs = nc.dram_tensor("hs_dump", [128, T + 1, 16], f16,
                              kind="ExternalOutput")
        d_sxd = nc.dram_tensor("sx_dump", [128, BL, N], f16,
                               kind="ExternalOutput")
        d_alp = nc.dram_tensor("alpha_dump", [4, 128, N], f16,
                               kind="ExternalOutput")
        d_amx = nc.dram_tensor("addmx_dump", [128, 48, T], f16,
                               kind="ExternalOutput")
        d_xhd = nc.dram_tensor("xh_dump", [128, 16, T], f16,
                               kind="ExternalOutput")

    LAG_E = 8    # e-tile tanh lag behind the scan
    LAG_S = 12   # score/softmax lag (slack lets the scheduler fill ACT gaps)

    with tile.TileContext(nc) as tc:
        with (
            tc.tile_pool(name="const", bufs=1) as cpool,
            tc.tile_pool(name="work", bufs=4) as wpool,
            tc.tile_pool(name="ebuf", bufs=4) as epool,
            tc.tile_pool(name="hpbuf", bufs=3) as hppool,
            tc.tile_pool(name="mh", bufs=2, space="PSUM") as mhpool,
            tc.tile_pool(name="mhh", bufs=2, space="PSUM") as mhhpool,
            tc.tile_pool(name="bigps", bufs=2, space="PSUM") as bpool,
            tc.tile_pool(name="hpps", bufs=2, space="PSUM") as hpspool,
        ):
            # ---- persistent tiles ----
            t_R8 = cpool.tile([128, 4, 8, 128], f8)
            t_Rh = cpool.tile([128, 4, 4, 128], f16)
            t_K = cpool.tile([128, 2, 12, 128], f16)
            t_w1 = cpool.tile([128, 128], f16)
            t_w2 = cpool.tile([128, 4, 128], f16)
            t_vbuf = cpool.tile([128, 257], f16)
            t_ident = cpool.tile([128, 128], f16)
            t_bzr = cpool.tile([128, 8], f32)
            t_bh = cpool.tile([128, 4], f32)
            t_bu = cpool.tile([128, 1], f32)
            t_d16 = [cpool.tile([128, N], f16, tag=f"d16_{b}", name=f"d16_{b}")
                     for b in range(BL)]
            t_dT = cpool.tile([128, 2, BL, 128], f16)      # dataT [p, nc, b, t]
            t_h0 = cpool.tile([BL, H], f16)
            t_addmx = cpool.tile([128, 48, T], f16)        # [mx_zr' | b_rec_h] per t
            t_xh = cpool.tile([128, 16, T], f16)           # xh' per t
            t_sx = cpool.tile([128, BL, N], f16)           # score_x' per b
            t_hs = cpool.tile([128, T + 1, 16], f16)       # h^T packed, slot t+1 = hs[t]
            t_alpha = [cpool.tile([128, N], f16, tag=f"alpha_{k}", name=f"alpha_{k}")
                       for k in range(4)]
            t_ssum = cpool.tile([128, 1], f32)
            t_rinv = cpool.tile([128, 1], f32)

            # ---- DMA in ----
            for b in range(BL):
                nc.sync.dma_start(out=t_d16[b][:, :], in_=d_data16.ap()[b, :, :])
            nc.sync.dma_start(out=t_R8[:, :, :, :], in_=d_R8.ap()[:, :, :, :])
            nc.sync.dma_start(out=t_Rh[:, :, :, :], in_=d_Rh.ap()[:, :, :, :])
            nc.sync.dma_start(out=t_K[:, :, :, :], in_=d_K.ap()[:, :, :, :])
            nc.sync.dma_start(out=t_w1[:, :], in_=d_w1.ap()[:, :])
            nc.sync.dma_start(out=t_w2[:, :, :], in_=d_w2.ap()[:, :, :])
            nc.sync.dma_start(out=t_vbuf[:, :], in_=d_vbuf.ap()[:, :])
            nc.sync.dma_start(out=t_ident[:, :], in_=d_ident.ap()[:, :])
            nc.sync.dma_start(out=t_bzr[:, :], in_=d_bzr.ap()[:, :])
            nc.sync.dma_start(out=t_bh[:, :], in_=d_bh.ap()[:, :])
            nc.sync.dma_start(out=t_addmx[:, 32:48, :], in_=d_brech.ap()[:, :, :])
            nc.sync.dma_start(out=t_bu[:, :], in_=d_bu.ap()[:, :])
            nc.sync.dma_start(out=t_h0[:, :], in_=d_h016.ap()[:, :])

            # ---- prologue: h0^T into hs slot 0 ----
            for j in range(4):
                ps = bpool.tile([128, 128], f16, tag="bigps")
                nc.tensor.transpose(ps[:, 0:BL], t_h0[0:BL, 128 * j:128 * (j + 1)],
                                    t_ident[0:BL, 0:BL])
                nc.vector.tensor_copy(t_hs[:, 0, 4 * j:4 * j + 4], ps[:, 0:BL])

            # ---- prologue: data^T  [p, nc, b, t] ----
            for b in range(BL):
                for n2 in range(2):
                    ps = bpool.tile([128, 128], f16, tag="bigps")
                    nc.tensor.transpose(ps[:, :], t_d16[b][:, 128 * n2:128 * (n2 + 1)],
                                        t_ident[:, :])
                    nc.vector.tensor_copy(t_dT[:, n2, b, :], ps[:, :])

            # ---- prologue: mx = data @ K (+biases), scattered per t ----
            for uc in range(12):
                ps = bpool.tile([128, BL, 128], f32, tag="bigps")
                for n2 in range(2):
                    nc.tensor.matmul(ps[:, :, :], t_K[:, n2, uc, :],
                                     t_dT[:, n2, :, :],
                                     start=(n2 == 0), stop=(n2 == 1))
                g, j = divmod(uc, 4)
                # psum free order is (b, t); dest free dims (b-within-col, t)
                if g < 2:
                    nc.scalar.activation(
                        t_addmx[:, 4 * uc:4 * uc + 4, :], ps[:, :, :],
                        Act.Identity, bias=t_bzr[:, uc:uc + 1])
                else:
                    nc.scalar.activation(
                        t_xh[:, 4 * j:4 * j + 4, :], ps[:, :, :],
                        Act.Identity, bias=t_bh[:, j:j + 1])

            # ---- prologue: score_x' ----
            for b in range(BL):
                ps = bpool.tile([128, N], f32, tag="bigps")
                nc.tensor.matmul(ps[:, :], t_w1[:, :], t_d16[b][:, :],
                                 start=True, stop=True)
                nc.scalar.activation(t_sx[:, b, :], ps[:, :],
                                     Act.Identity, bias=t_bu[:, :])

            # ---- helpers for the lagged attention pipeline ----
            hp_tiles = {}  # block index -> sbuf tile [128, 32] f32

            def emit_hp_block(blk):
                t0 = 8 * blk
                ps = hpspool.tile([128, 32], f32)
                for hc in range(4):
                    nc.tensor.matmul(ps[:, :], t_w2[:, hc, :],
                                     t_hs[:, t0 + 1:t0 + 9, 4 * hc:4 * hc + 4],
                                     start=(hc == 0), stop=(hc == 3))
                hp = hppool.tile([128, 32], f32)
                nc.vector.tensor_copy(hp[:, :], ps[:, :])
                hp_tiles[blk] = hp

            e_tiles = {}  # ta -> e tile [128, BL, N] f16

            def emit_e(ta):
                hp = hp_tiles[ta // 8]
                tl = ta % 8
                ei = epool.tile([128, BL, N], f16, tag="ein")
                for b in range(BL):
                    nc.vector.tensor_scalar_add(ei[:, b, :], t_sx[:, b, :],
                                                hp[:, 4 * tl + b:4 * tl + b + 1])
                et = epool.tile([128, BL, N], f16, tag="etile", bufs=6)
                nc.scalar.activation(et[:, :, :], ei[:, :, :], Act.Tanh)
                e_tiles[ta] = et

            score_ps = {}  # group (8 steps) -> psum tile [128, N] f32

            def emit_score_mm(ta):
                et = e_tiles.pop(ta)
                g, tl = divmod(ta, 8)
                if tl == 0:
                    score_ps[g] = bpool.tile([128, N], f32, tag="bigps",
                                             name=f"scps_{g}")
                ps = score_ps[g]
                q = (ta % 32) // 8          # 32-aligned row group within psum
                for l in range(BL):
                    c = 4 * tl + l          # column within the 32-wide window
                    nc.tensor.matmul(ps[32 * q:32 * q + 32, :],
                                     t_vbuf[:, 128 - c:160 - c], et[:, l, :],
                                     start=(tl == 0 and l == 0),
                                     stop=(tl == 7 and l == BL - 1),
                                     skip_group_check=True,
                                     tile_position=(0, 32 * q))

            def emit_softmax_group(g):
                ps = score_ps.pop(g)
                p0 = 32 * (g % 4)
                ex = wpool.tile([128, N], f16, tag="expv")
                nc.scalar.activation(ex[p0:p0 + 32, :], ps[p0:p0 + 32, :], Act.Exp,
                                     accum_out=t_ssum[p0:p0 + 32, :])
                nc.vector.reciprocal(t_rinv[p0:p0 + 32, :], t_ssum[p0:p0 + 32, :])
                nc.vector.tensor_scalar(
                    t_alpha[g // 4][p0:p0 + 32, :], ex[p0:p0 + 32, :],
                    t_rinv[p0:p0 + 32, :], None, Alu.mult)

            # ---- main scan loop ----
            for t in range(T):
                if t % 8 == 0 and t >= 8:
                    emit_hp_block(t // 8 - 1)

                mh = mhpool.tile([128, 32], f32)
                mhh = mhhpool.tile([128, 16], f32)
                # bias/mx seeds: start=True clears each bank
                nc.tensor.matmul(mh[:, 0:32], t_ident[:, :], t_addmx[:, 0:32, t],
                                 start=True, stop=False, skip_group_check=True)
                for uc in range(8):
                    for kc in range(4):
                        nc.tensor.matmul(mh[:, 4 * uc:4 * uc + 4],
                                         t_R8[:, kc, uc, :],
                                         t_hs[:, t, 4 * kc:4 * kc + 4],
                                         start=False,
                                         stop=(uc == 7 and kc == 3),
                                         skip_group_check=True)
                nc.tensor.matmul(mhh[:, 0:16], t_ident[:, :], t_addmx[:, 32:48, t],
                                 start=True, stop=False, skip_group_check=True)
                for uc in range(4):
                    for kc in range(4):
                        nc.tensor.matmul(mhh[:, 4 * uc:4 * uc + 4],
                                         t_Rh[:, kc, uc, :],
                                         t_hs[:, t, 4 * kc:4 * kc + 4],
                                         start=False,
                                         stop=(uc == 3 and kc == 3),
                                         skip_group_check=True)

                # gates: sigma(x) = (1 + tanh(x/2))/2 folded into STT ops
                tz = wpool.tile([128, 16], f16, tag="tz")
                tr = wpool.tile([128, 16], f16, tag="tr")
                t2 = wpool.tile([128, 16], f16, tag="t2")
                t3 = wpool.tile([128, 16], f16, tag="t3")
                cg = wpool.tile([128, 16], f16, tag="cg")
                dg = wpool.tile([128, 16], f16, tag="dg")
                mg = wpool.tile([128, 16], f16, tag="mg")

                nc.scalar.activation(tr[:, :], mh[:, 16:32], Act.Tanh, scale=0.5)
                # t2 = (tr + 1) * (hh + b_rec_h);  t3 = 0.5 t2 + xh
                nc.vector.scalar_tensor_tensor(t2[:, :], tr[:, :], 1.0,
                                               mhh[:, 0:16], Alu.add, Alu.mult)
                nc.vector.scalar_tensor_tensor(t3[:, :], t2[:, :], 0.5,
                                               t_xh[:, :, t], Alu.mult, Alu.add)
                nc.scalar.activation(cg[:, :], t3[:, :], Act.Tanh)
                nc.scalar.activation(tz[:, :], mh[:, 0:16], Act.Tanh, scale=0.5)
                nc.vector.tensor_tensor(dg[:, :], t_hs[:, t, :], cg[:, :],
                                        Alu.subtract)
                # m = (tz + 1) * d;  h_new = 0.5 m + c
                nc.vector.scalar_tensor_tensor(mg[:, :], tz[:, :], 1.0,
                                               dg[:, :], Alu.add, Alu.mult)
                nc.vector.scalar_tensor_tensor(t_hs[:, t + 1, :], mg[:, :], 0.5,
                                               cg[:, :], Alu.mult, Alu.add)

                # lagged attention
                if SCAN_ONLY:
                    continue
                if t >= LAG_E:
                    emit_e(t - LAG_E)
                if t >= LAG_S:
                    emit_score_mm(t - LAG_S)
                    if (t - LAG_S) % 8 == 7:
                        emit_softmax_group((t - LAG_S) // 8)

            # ---- attention epilogue ----
            if not SCAN_ONLY:
                emit_hp_block(15)
            if not SCAN_ONLY:
                for ta in range(T - LAG_E, T):
                    emit_e(ta)
                for ta in range(T - LAG_S, T):
                    emit_score_mm(ta)
                    if ta % 8 == 7:
                        emit_softmax_group(ta // 8)

            if DEBUG:
                nc.sync.dma_start(out=d_hs.ap()[:, :, :], in_=t_hs[:, :, :])
                nc.sync.dma_start(out=d_sxd.ap()[:, :, :], in_=t_sx[:, :, :])
                for k in range(4):
                    nc.sync.dma_start(out=d_alp.ap()[k, :, :],
                                      in_=t_alpha[k][:, :])
                nc.sync.dma_start(out=d_amx.ap()[:, :, :], in_=t_addmx[:, :, :])
                nc.sync.dma_start(out=d_xhd.ap()[:, :, :], in_=t_xh[:, :, :])

            # ---- final out = data * alpha ----
            for k in range(4):
                dt_ = wpool.tile([128, N], f32, tag="dmul")
                ot = wpool.tile([128, N], f32, tag="omul")
                nc.sync.dma_start(out=dt_[:, :], in_=d_dataout.ap()[k, :, :])
                amul = dt_ if SCAN_ONLY else t_alpha[k]
                nc.vector.tensor_tensor(ot[:, :], dt_[:, :], amul[:, :],
                                        Alu.mult)
                nc.sync.dma_start(out=d_out.ap()[k, :, :], in_=ot[:, :])

    nc.compile()
    return nc


def _prep_inputs(data, h0, gru_kernel, gru_rkernel, gru_bias,
                 w1_w, w1_b, w2_w, w2_b, v_w, v_b):
    f16 = np.float16
    f32 = np.float32

    import ml_dtypes
    R_all = np.ascontiguousarray(
        gru_rkernel.reshape(4, 128, 12, 128).transpose(1, 0, 2, 3))
    R8_l = R_all[:, :, 0:8, :].astype(ml_dtypes.float8_e4m3)
    Rh_l = R_all[:, :, 8:12, :].astype(f16)
    K16 = gru_kernel.astype(f16)                      # (256, 1536)
    K_l = np.ascontiguousarray(
        K16.reshape(2, 128, 12, 128).transpose(1, 0, 2, 3))
    w1_l = w1_w.astype(f16)                           # (128, 128)
    w2_l = np.ascontiguousarray(
        w2_w.astype(f16).reshape(4, 128, 128).transpose(1, 0, 2))
    vbuf = np.zeros((128, 257), f16)
    vbuf[:, 128] = v_w[:, 0].astype(f16)
    ident = np.eye(128, dtype=f16)

    b_in, b_rec = gru_bias[0].astype(f32), gru_bias[1].astype(f32)
    bzr = (b_in + b_rec)[:1024].reshape(8, 128).T.copy()      # [128, 8]
    bh = b_in[1024:].reshape(4, 128).T.copy()                 # [128, 4]
    # brech16[p, 4j+b] = b_rec[1024 + 128 j + p], replicated along t
    brech16 = np.zeros((128, 16), f16)
    for j in range(4):
        for bb in range(4):
            brech16[:, 4 * j + bb] = b_rec[1024 + 128 * j:1024 + 128 * (j + 1)]
    brech_rep = np.ascontiguousarray(
        np.repeat(brech16[:, :, None], T, axis=2))
    bu = (w1_b + w2_b).astype(f32).reshape(128, 1)

    data16 = data.astype(f16)
    h016 = h0.astype(f16)

    per_core = []
    for c in range(NC):
        sl = slice(BL * c, BL * (c + 1))
        # rows (b, i, l): t = 32 i + 4 c + l
        bidx = np.repeat(np.arange(B), 16)
        tidx = (32 * np.tile(np.repeat(np.arange(4), 4), B)
                + 4 * c + np.tile(np.arange(4), B * 4))
        dataout = data[bidx, tidx, :].astype(f32).reshape(4, 128, N)
        per_core.append({
            "data16": data16[sl], "dataout": dataout, "h016": h016[sl],
            "R8_l": R8_l, "Rh_l": Rh_l, "K_l": K_l, "w1_l": w1_l, "w2_l": w2_l,
            "vbuf": vbuf, "ident": ident, "bias_zr": bzr, "bias_h": bh,
            "brech_rep": brech_rep, "bias_u": bu,
        })
    return per_core


def kernel(**inputs):
    from concourse.bass_utils import run_bass_kernel_spmd

    if "nc" not in _CACHE:
        _CACHE["nc"] = _build()
    nc = _CACHE["nc"]

    args = {k: np.asarray(v) for k, v in inputs.items()}
    per_core = _prep_inputs(
        args["data"], args["h0"], args["gru_kernel"], args["gru_rkernel"],
        args["gru_bias"], args["w1_w"], args["w1_b"], args["w2_w"],
        args["w2_b"], args["v_w"], args["v_b"])

    if "warm" not in _CACHE:
        # first execution after NEFF load can race; discard it
        run_bass_kernel_spmd(nc, per_core, core_ids=list(range(NC)))
        _CACHE["warm"] = True
    res = run_bass_kernel_spmd(nc, per_core, core_ids=list(range(NC)))
    _CACHE["last_res"] = res

    out = np.empty((B, T, N), np.float32)
    for c in range(NC):
        o = res.results[c]["out"].reshape(512, N)
        bidx = np.repeat(np.arange(B), 16)
        tidx = (32 * np.tile(np.repeat(np.arange(4), 4), B)
                + 4 * c + np.tile(np.arange(4), B * 4))
        out[bidx, tidx, :] = o
    return out


# revision 39
# speedup vs baseline: 1.2910x; 1.0299x over previous
"""Trainium2 Bass kernel for nn_Encoder (GRU + input attention).

Shapes (hardcoded): B=32, T=128, N=256, H=512; 8 NeuronCores, batch
sharded 4 examples/core.

Math (matching the reference):
  hs = GRU scan over T steps (Keras GRUCell, reset_after=True, gates z,r,h)
  score_x[b,n,u] = sum_t data[b,t,n] w1_w[t,u] + w1_b[u]
  hp[t,b,u]     = hs[t,b,:] @ w2_w + w2_b[u]
  score[t,b,n]  = sum_u v[u] tanh(score_x[b,n,u] + hp[t,b,u])   (+v_b: softmax-invariant)
  alpha = softmax_n(score);  out[b,t,:] = data[b,t,:] * alpha[(b*T+t)//B, (b*T+t)%B, :]

Per-core layout: u (or H-chunks) on partitions. The recurrent matmul keeps
R chunks stationary (fp16, FWL) and streams h^T (128,4) slices; gate adds for
z/r are folded into the PSUM accumulation via an identity-matmul; sigmoid is
computed as (1+tanh(x/2))/2 so tanh+exp live in one activation table set.
"""

import numpy as np

B, T, N, H = 32, 128, 256, 512
NC = 8           # cores
BL = B // NC     # batch per core (4)
H3 = 3 * H

_CACHE = {}
DEBUG = False
import os
SCAN_ONLY = os.environ.get("SCAN_ONLY", "0") == "1"


def _build():
    import concourse.bass as bass
    import concourse.bacc as bacc
    import concourse.tile as tile
    import concourse.mybir as mybir

    f16 = mybir.dt.float16
    f32 = mybir.dt.float32
    Alu = mybir.AluOpType
    Act = mybir.ActivationFunctionType

    nc = bacc.Bacc("TRN2", target_bir_lowering=False, debug=False)

    # ---- dram I/O ----
    d_data16 = nc.dram_tensor("data16", [BL, T, N], f16, kind="ExternalInput")
    d_dataout = nc.dram_tensor("dataout", [4, 128, N], f32, kind="ExternalInput")
    d_h016 = nc.dram_tensor("h016", [BL, H], f16, kind="ExternalInput")
    f8 = mybir.dt.float8e4
    d_R8 = nc.dram_tensor("R8_l", [128, 4, 8, 128], f8, kind="ExternalInput")
    d_Rh = nc.dram_tensor("Rh_l", [128, 4, 4, 128], f16, kind="ExternalInput")
    d_K = nc.dram_tensor("K_l", [128, 2, 12, 128], f16, kind="ExternalInput")
    d_w1 = nc.dram_tensor("w1_l", [128, 128], f16, kind="ExternalInput")
    d_w2 = nc.dram_tensor("w2_l", [128, 4, 128], f16, kind="ExternalInput")
    d_vbuf = nc.dram_tensor("vbuf", [128, 257], f16, kind="ExternalInput")
    d_ident = nc.dram_tensor("ident", [128, 128], f16, kind="ExternalInput")
    d_bzr = nc.dram_tensor("bias_zr", [128, 8], f32, kind="ExternalInput")
    d_bh = nc.dram_tensor("bias_h", [128, 4], f32, kind="ExternalInput")
    d_brech = nc.dram_tensor("brech_rep", [128, 16, T], f16, kind="ExternalInput")
    d_bu = nc.dram_tensor("bias_u", [128, 1], f32, kind="ExternalInput")
    d_out = nc.dram_tensor("out", [4, 128, N], f32, kind="ExternalOutput")
    if DEBUG:
        d_hs = nc.dram_tensor("hs_dump", [128, T + 1, 16], f16,
                              kind="ExternalOutput")
        d_sxd = nc.dram_tensor("sx_dump", [128, BL, N], f16,
                               kind="ExternalOutput")
        d_alp = nc.dram_tensor("alpha_dump", [4, 128, N], f16,
                               kind="ExternalOutput")
        d_amx = nc.dram_tensor("addmx_dump", [128, 48, T], f16,
                               kind="ExternalOutput")
        d_xhd = nc.dram_tensor("xh_dump", [128, 16, T], f16,
                               kind="ExternalOutput")

    LAG_E = 8    # e-tile tanh lag behind the scan
    LAG_S = 12   # score/softmax lag (slack lets the scheduler fill ACT gaps)

    with tile.TileContext(nc) as tc:
        with (
            tc.tile_pool(name="const", bufs=1) as cpool,
            tc.tile_pool(name="work", bufs=4) as wpool,
            tc.tile_pool(name="ebuf", bufs=4) as epool,
            tc.tile_pool(name="hpbuf", bufs=3) as hppool,
            tc.tile_pool(name="mh", bufs=2, space="PSUM") as mhpool,
            tc.tile_pool(name="mhh", bufs=2, space="PSUM") as mhhpool,
            tc.tile_pool(name="bigps", bufs=2, space="PSUM") as bpool,
            tc.tile_pool(name="hpps", bufs=2, space="PSUM") as hpspool,
        ):
            # ---- persistent tiles ----
            t_R8 = cpool.tile([128, 4, 8, 128], f8)
            t_Rh = cpool.tile([128, 4, 4, 128], f16)
            t_K = cpool.tile([128, 2, 12, 128], f16)
            t_w1 = cpool.tile([128, 128], f16)
            t_w2 = cpool.tile([128, 4, 128], f16)
            t_vbuf = cpool.tile([128, 257], f16)
            t_ident = cpool.tile([128, 128], f16)
            t_bzr = cpool.tile([128, 8], f32)
            t_bh = cpool.tile([128, 4], f32)
            t_bu = cpool.tile([128, 1], f32)
            t_d16 = [cpool.tile([128, N], f16, tag=f"d16_{b}", name=f"d16_{b}")
                     for b in range(BL)]
            t_dT = cpool.tile([128, 2, BL, 128], f16)      # dataT [p, nc, b, t]
            t_h0 = cpool.tile([BL, H], f16)
            t_addmx = cpool.tile([128, 48, T], f16)        # [mx_zr' | b_rec_h] per t
            t_xh = cpool.tile([128, 16, T], f16)           # xh' per t
            t_sx = cpool.tile([128, BL, N], f16)           # score_x' per b
            t_hs = cpool.tile([128, T + 1, 16], f16)       # h^T packed, slot t+1 = hs[t]
            t_alpha = [cpool.tile([128, N], f16, tag=f"alpha_{k}", name=f"alpha_{k}")
                       for k in range(4)]
            t_ssum = cpool.tile([128, 1], f32)
            t_rinv = cpool.tile([128, 1], f32)

            # ---- DMA in ----
            for b in range(BL):
                nc.sync.dma_start(out=t_d16[b][:, :], in_=d_data16.ap()[b, :, :])
            nc.sync.dma_start(out=t_R8[:, :, :, :], in_=d_R8.ap()[:, :, :, :])
            nc.sync.dma_start(out=t_Rh[:, :, :, :], in_=d_Rh.ap()[:, :, :, :])
            nc.sync.dma_start(out=t_K[:, :, :, :], in_=d_K.ap()[:, :, :, :])
            nc.sync.dma_start(out=t_w1[:, :], in_=d_w1.ap()[:, :])
            nc.sync.dma_start(out=t_w2[:, :, :], in_=d_w2.ap()[:, :, :])
            nc.sync.dma_start(out=t_vbuf[:, :], in_=d_vbuf.ap()[:, :])
            nc.sync.dma_start(out=t_ident[:, :], in_=d_ident.ap()[:, :])
            nc.sync.dma_start(out=t_bzr[:, :], in_=d_bzr.ap()[:, :])
            nc.sync.dma_start(out=t_bh[:, :], in_=d_bh.ap()[:, :])
            nc.sync.dma_start(out=t_addmx[:, 32:48, :], in_=d_brech.ap()[:, :, :])
            nc.sync.dma_start(out=t_bu[:, :], in_=d_bu.ap()[:, :])
            nc.sync.dma_start(out=t_h0[:, :], in_=d_h016.ap()[:, :])

            # ---- prologue: h0^T into hs slot 0 ----
            for j in range(4):
                ps = bpool.tile([128, 128], f16, tag="bigps")
                nc.tensor.transpose(ps[:, 0:BL], t_h0[0:BL, 128 * j:128 * (j + 1)],
                                    t_ident[0:BL, 0:BL])
                nc.vector.tensor_copy(t_hs[:, 0, 4 * j:4 * j + 4], ps[:, 0:BL])

            # ---- prologue: data^T  [p, nc, b, t] ----
            for b in range(BL):
                for n2 in range(2):
                    ps = bpool.tile([128, 128], f16, tag="bigps")
                    nc.tensor.transpose(ps[:, :], t_d16[b][:, 128 * n2:128 * (n2 + 1)],
                                        t_ident[:, :])
                    nc.vector.tensor_copy(t_dT[:, n2, b, :], ps[:, :])

            # ---- prologue: mx = data @ K (+biases), scattered per t ----
            for uc in range(12):
                ps = bpool.tile([128, BL, 128], f32, tag="bigps")
                for n2 in range(2):
                    nc.tensor.matmul(ps[:, :, :], t_K[:, n2, uc, :],
                                     t_dT[:, n2, :, :],
                                     start=(n2 == 0), stop=(n2 == 1))
                g, j = divmod(uc, 4)
                # psum free order is (b, t); dest free dims (b-within-col, t)
                if g < 2:
                    nc.scalar.activation(
                        t_addmx[:, 4 * uc:4 * uc + 4, :], ps[:, :, :],
                        Act.Identity, bias=t_bzr[:, uc:uc + 1])
                else:
                    nc.scalar.activation(
                        t_xh[:, 4 * j:4 * j + 4, :], ps[:, :, :],
                        Act.Identity, bias=t_bh[:, j:j + 1])

            # ---- prologue: score_x' ----
            for b in range(BL):
                ps = bpool.tile([128, N], f32, tag="bigps")
                nc.tensor.matmul(ps[:, :], t_w1[:, :], t_d16[b][:, :],
                                 start=True, stop=True)
                nc.scalar.activation(t_sx[:, b, :], ps[:, :],
                                     Act.Identity, bias=t_bu[:, :])

            # ---- helpers for the lagged attention pipeline ----
            hp_tiles = {}  # block index -> sbuf tile [128, 32] f32

            def emit_hp_block(blk):
                t0 = 8 * blk
                ps = hpspool.tile([128, 32], f32)
                for hc in range(4):
                    nc.tensor.matmul(ps[:, :], t_w2[:, hc, :],
                                     t_hs[:, t0 + 1:t0 + 9, 4 * hc:4 * hc + 4],
                                     start=(hc == 0), stop=(hc == 3))
                hp = hppool.tile([128, 32], f32)
                nc.vector.tensor_copy(hp[:, :], ps[:, :])
                hp_tiles[blk] = hp

            e_tiles = {}  # ta -> e tile [128, BL, N] f16

            def emit_e(ta):
                hp = hp_tiles[ta // 8]
                tl = ta % 8
                ei = epool.tile([128, BL, N], f16, tag="ein")
                for b in range(BL):
                    nc.vector.tensor_scalar_add(ei[:, b, :], t_sx[:, b, :],
                                                hp[:, 4 * tl + b:4 * tl + b + 1])
                et = epool.tile([128, BL, N], f16, tag="etile", bufs=6)
                nc.scalar.activation(et[:, 0:2, :], ei[:, 0:2, :], Act.Tanh)
                nc.scalar.activation(et[:, 2:4, :], ei[:, 2:4, :], Act.Tanh)
                e_tiles[ta] = et

            score_ps = {}  # group (8 steps) -> psum tile [128, N] f32

            def emit_score_mm(ta):
                et = e_tiles.pop(ta)
                g, tl = divmod(ta, 8)
                if tl == 0:
                    score_ps[g] = bpool.tile([128, N], f32, tag="bigps",
                                             name=f"scps_{g}")
                ps = score_ps[g]
                q = (ta % 32) // 8          # 32-aligned row group within psum
                for l in range(BL):
                    c = 4 * tl + l          # column within the 32-wide window
                    nc.tensor.matmul(ps[32 * q:32 * q + 32, :],
                                     t_vbuf[:, 128 - c:160 - c], et[:, l, :],
                                     start=(tl == 0 and l == 0),
                                     stop=(tl == 7 and l == BL - 1),
                                     skip_group_check=True,
                                     tile_position=(0, 32 * q))

            def emit_softmax_group(g):
                ps = score_ps.pop(g)
                p0 = 32 * (g % 4)
                ex = wpool.tile([128, N], f16, tag="expv")
                nc.scalar.activation(ex[p0:p0 + 32, :], ps[p0:p0 + 32, :], Act.Exp,
                                     accum_out=t_ssum[p0:p0 + 32, :])
                nc.vector.reciprocal(t_rinv[p0:p0 + 32, :], t_ssum[p0:p0 + 32, :])
                nc.vector.tensor_scalar(
                    t_alpha[g // 4][p0:p0 + 32, :], ex[p0:p0 + 32, :],
                    t_rinv[p0:p0 + 32, :], None, Alu.mult)

            # ---- main scan loop ----
            for t in range(T):
                if t % 8 == 0 and t >= 8:
                    emit_hp_block(t // 8 - 1)

                mh = mhpool.tile([128, 32], f32)
                mhh = mhhpool.tile([128, 16], f32)
                # bias/mx seeds: start=True clears each bank
                nc.tensor.matmul(mh[:, 0:32], t_ident[:, :], t_addmx[:, 0:32, t],
                                 start=True, stop=False, skip_group_check=True)
                for uc in range(8):
                    for kc in range(4):
                        nc.tensor.matmul(mh[:, 4 * uc:4 * uc + 4],
                                         t_R8[:, kc, uc, :],
                                         t_hs[:, t, 4 * kc:4 * kc + 4],
                                         start=False,
                                         stop=(uc == 7 and kc == 3),
                                         skip_group_check=True)
                nc.tensor.matmul(mhh[:, 0:16], t_ident[:, :], t_addmx[:, 32:48, t],
                                 start=True, stop=False, skip_group_check=True)
                for uc in range(4):
                    for kc in range(4):
                        nc.tensor.matmul(mhh[:, 4 * uc:4 * uc + 4],
                                         t_Rh[:, kc, uc, :],
                                         t_hs[:, t, 4 * kc:4 * kc + 4],
                                         start=False,
                                         stop=(uc == 3 and kc == 3),
                                         skip_group_check=True)

                # gates: sigma(x) = (1 + tanh(x/2))/2 folded into STT ops
                tz = wpool.tile([128, 16], f16, tag="tz")
                tr = wpool.tile([128, 16], f16, tag="tr")
                t2 = wpool.tile([128, 16], f16, tag="t2")
                t3 = wpool.tile([128, 16], f16, tag="t3")
                cg = wpool.tile([128, 16], f16, tag="cg")

                nc.scalar.activation(tr[:, :], mh[:, 16:32], Act.Tanh, scale=0.5)
                # t2 = (tr + 1) * (hh + b_rec_h);  t3 = 0.5 t2 + xh
                nc.vector.scalar_tensor_tensor(t2[:, :], tr[:, :], 1.0,
                                               mhh[:, 0:16], Alu.add, Alu.mult)
                nc.vector.scalar_tensor_tensor(t3[:, :], t2[:, :], 0.5,
                                               t_xh[:, :, t], Alu.mult, Alu.add)
                nc.scalar.activation(tz[:, :], mh[:, 0:16], Act.Tanh, scale=0.5)
                # parallel with the c-path: z = (tz+1)/2, zh = z*h, s = 1-z
                zt = wpool.tile([128, 16], f16, tag="zt")
                uh = wpool.tile([128, 16], f16, tag="uh")
                st = wpool.tile([128, 16], f16, tag="st")
                sc = wpool.tile([128, 16], f16, tag="sc")
                nc.vector.tensor_scalar(zt[:, :], tz[:, :], 1.0, 0.5,
                                        Alu.add, Alu.mult)
                nc.vector.tensor_tensor(uh[:, :], zt[:, :], t_hs[:, t, :],
                                        Alu.mult)
                nc.vector.tensor_scalar(st[:, :], tz[:, :], -0.5, 0.5,
                                        Alu.mult, Alu.add)
                nc.scalar.activation(cg[:, :], t3[:, :], Act.Tanh)
                # post-c tail: h_new = zh + (1-z)*c
                nc.vector.tensor_tensor(sc[:, :], st[:, :], cg[:, :], Alu.mult)
                nc.vector.tensor_tensor(t_hs[:, t + 1, :], uh[:, :], sc[:, :],
                                        Alu.add)

                # lagged attention
                if t >= LAG_E and not SCAN_ONLY:
                    emit_e(t - LAG_E)
                if t >= LAG_S and not SCAN_ONLY:
                    emit_score_mm(t - LAG_S)
                    if (t - LAG_S) % 8 == 7:
                        emit_softmax_group((t - LAG_S) // 8)


            # ---- attention epilogue ----
            if not SCAN_ONLY:
                emit_hp_block(15)
            if not SCAN_ONLY:
                for ta in range(T - LAG_E, T):
                    emit_e(ta)
                for ta in range(T - LAG_S, T):
                    emit_score_mm(ta)
                    if ta % 8 == 7:
                        emit_softmax_group(ta // 8)

            if DEBUG:
                nc.sync.dma_start(out=d_hs.ap()[:, :, :], in_=t_hs[:, :, :])
                nc.sync.dma_start(out=d_sxd.ap()[:, :, :], in_=t_sx[:, :, :])
                for k in range(4):
                    nc.sync.dma_start(out=d_alp.ap()[k, :, :],
                                      in_=t_alpha[k][:, :])
                nc.sync.dma_start(out=d_amx.ap()[:, :, :], in_=t_addmx[:, :, :])
                nc.sync.dma_start(out=d_xhd.ap()[:, :, :], in_=t_xh[:, :, :])

            # ---- final out = data * alpha ----
            for k in range(4):
                dt_ = wpool.tile([128, N], f32, tag="dmul")
                ot = wpool.tile([128, N], f32, tag="omul")
                nc.sync.dma_start(out=dt_[:, :], in_=d_dataout.ap()[k, :, :])
                amul = dt_ if SCAN_ONLY else t_alpha[k]
                nc.vector.tensor_tensor(ot[:, :], dt_[:, :], amul[:, :],
                                        Alu.mult)
                nc.sync.dma_start(out=d_out.ap()[k, :, :], in_=ot[:, :])

    nc.compile()
    return nc


def _prep_inputs(data, h0, gru_kernel, gru_rkernel, gru_bias,
                 w1_w, w1_b, w2_w, w2_b, v_w, v_b):
    f16 = np.float16
    f32 = np.float32

    import ml_dtypes
    R_all = np.ascontiguousarray(
        gru_rkernel.reshape(4, 128, 12, 128).transpose(1, 0, 2, 3))
    R8_l = R_all[:, :, 0:8, :].astype(ml_dtypes.float8_e4m3)
    Rh_l = R_all[:, :, 8:12, :].astype(f16)
    K16 = gru_kernel.astype(f16)                      # (256, 1536)
    K_l = np.ascontiguousarray(
        K16.reshape(2, 128, 12, 128).transpose(1, 0, 2, 3))
    w1_l = w1_w.astype(f16)                           # (128, 128)
    w2_l = np.ascontiguousarray(
        w2_w.astype(f16).reshape(4, 128, 128).transpose(1, 0, 2))
    vbuf = np.zeros((128, 257), f16)
    vbuf[:, 128] = v_w[:, 0].astype(f16)
    ident = np.eye(128, dtype=f16)

    b_in, b_rec = gru_bias[0].astype(f32), gru_bias[1].astype(f32)
    bzr = (b_in + b_rec)[:1024].reshape(8, 128).T.copy()      # [128, 8]
    bh = b_in[1024:].reshape(4, 128).T.copy()                 # [128, 4]
    # brech16[p, 4j+b] = b_rec[1024 + 128 j + p], replicated along t
    brech16 = np.zeros((128, 16), f16)
    for j in range(4):
        for bb in range(4):
            brech16[:, 4 * j + bb] = b_rec[1024 + 128 * j:1024 + 128 * (j + 1)]
    brech_rep = np.ascontiguousarray(
        np.repeat(brech16[:, :, None], T, axis=2))
    bu = (w1_b + w2_b).astype(f32).reshape(128, 1)

    data16 = data.astype(f16)
    h016 = h0.astype(f16)

    per_core = []
    for c in range(NC):
        sl = slice(BL * c, BL * (c + 1))
        # rows (b, i, l): t = 32 i + 4 c + l
        bidx = np.repeat(np.arange(B), 16)
        tidx = (32 * np.tile(np.repeat(np.arange(4), 4), B)
                + 4 * c + np.tile(np.arange(4), B * 4))
        dataout = data[bidx, tidx, :].astype(f32).reshape(4, 128, N)
        per_core.append({
            "data16": data16[sl], "dataout": dataout, "h016": h016[sl],
            "R8_l": R8_l, "Rh_l": Rh_l, "K_l": K_l, "w1_l": w1_l, "w2_l": w2_l,
            "vbuf": vbuf, "ident": ident, "bias_zr": bzr, "bias_h": bh,
            "brech_rep": brech_rep, "bias_u": bu,
        })
    return per_core


def kernel(**inputs):
    from concourse.bass_utils import run_bass_kernel_spmd

    if "nc" not in _CACHE:
        _CACHE["nc"] = _build()
    nc = _CACHE["nc"]

    args = {k: np.asarray(v) for k, v in inputs.items()}
    per_core = _prep_inputs(
        args["data"], args["h0"], args["gru_kernel"], args["gru_rkernel"],
        args["gru_bias"], args["w1_w"], args["w1_b"], args["w2_w"],
        args["w2_b"], args["v_w"], args["v_b"])

    if "warm" not in _CACHE:
        # first execution after NEFF load can race; discard it
        run_bass_kernel_spmd(nc, per_core, core_ids=list(range(NC)))
        _CACHE["warm"] = True
    res = run_bass_kernel_spmd(nc, per_core, core_ids=list(range(NC)))
    _CACHE["last_res"] = res

    out = np.empty((B, T, N), np.float32)
    for c in range(NC):
        o = res.results[c]["out"].reshape(512, N)
        bidx = np.repeat(np.arange(B), 16)
        tidx = (32 * np.tile(np.repeat(np.arange(4), 4), B)
                + 4 * c + np.tile(np.arange(4), B * 4))
        out[bidx, tidx, :] = o
    return out
